# revision 46
# baseline (speedup 1.0000x reference)
"""Trainium2 Bass kernel for the dual-branch cross-attention block (N=4096, D=512).

Sharding: rows of Q / output across 8 cores (512 rows each). K and V shards are
computed locally and all-gathered; the column-sum renorm uses a tiny AllReduce.

v2 layout/schedule:
- linears in f32r; V / P-transpose storage / FFN path in bf16 (rel_err ~5e-3).
- V and the residual-MLP output are produced directly in natural (row-major)
  layout by using the activation tiles as the matmul stationary operand.
- Row softmax scale (1/rowsum) is applied to the exp'd scores on the scalar
  engine before the P transposes; column sums then come from cheap vector
  reduces over the transposed tiles (no tensor-engine colsum matmuls).
- Branches are interleaved (S_a, S_b, PV_a+epilogue_a, PV_b+epilogue_b) so both
  column-sum AllReduces hide under tensor work; V gathers are bf16.
"""
import numpy as np

try:
    import concourse.bass as bass  # noqa: F401
except ImportError:
    import sys
    sys.path.insert(0, "/opt/trn_rl_repo")
import concourse.bass as bass
import concourse.tile as tile
from concourse import bacc, mybir
from concourse import bass_utils
from concourse.masks import make_identity
import ml_dtypes

F32 = mybir.dt.float32
F32R = mybir.dt.float32r
BF16 = mybir.dt.bfloat16
AF = mybir.ActivationFunctionType
ALU = mybir.AluOpType
AX = mybir.AxisListType

N, D, R, NS, P = 4096, 512, 8, 512, 128
KT = D // P   # 4 d-tiles
IT = NS // P  # 4 i-tiles per core
NT = N // P   # 32 n-tiles global
RG = [list(range(R))]
EPS_LN, EPS_ATTN = 1e-6, 1e-9

WNAMES = ("pw1", "pw2", "rw1", "rw2", "rw3", "wq", "wk", "wv")
BNAMES = ("pb1", "pb2", "rb1", "rb2")


def _body(tc, ins, out_ext):
    nc = tc.nc

    import contextlib
    stack = contextlib.ExitStack()

    # ---- whole-kernel pools ----
    const = stack.enter_context(tc.tile_pool(name="const", bufs=1))
    dram = stack.enter_context(tc.tile_pool(name="dram", bufs=1, space="DRAM"))

    ident = const.tile([P, P], F32, tag="ident", bufs=1)
    make_identity(nc, ident[:])
    identr = const.tile([P, P], F32R, tag="identr", bufs=1)
    nc.vector.tensor_copy(identr[:], ident[:])
    identb = const.tile([P, P], BF16, tag="identb", bufs=1)
    nc.vector.tensor_copy(identb[:], ident[:])
    onesb = const.tile([P, 1], BF16, tag="onesb", bufs=1)
    nc.vector.memset(onesb[:], 1.0)
    epsln = const.tile([P, 1], F32, tag="epsln", bufs=1)
    nc.vector.memset(epsln[:], EPS_LN)

    # LN params (pre-replicated on host to [128, 512]); DMAs deferred past the
    # startup-critical x0/weight loads (lnp only needed in the epilogue).
    lnp = {}
    for p in ("a", "b"):
        for g in ("lng", "lnb"):
            lnp[p, g] = const.tile([P, D], F32, tag="lnp", bufs=4, name=f"lnp_{p}_{g}")
    # feature-major biases [128, 4]; rb3 broadcast [128, 512]
    bias = {}
    for p in ("a", "b"):
        for b in BNAMES + ("bk", "bq"):
            t = const.tile([P, KT], F32, tag="bias", bufs=12, name=f"bias_{p}_{b}")
            nc.sync.dma_start(t[:], ins[f"{p}_{b}"][:, :])
            bias[p, b] = t
        bias[p, "rb3bc"] = const.tile([P, D], F32, tag="b3bc", bufs=4, name=f"b3bc_{p}")
        bias[p, "bvbc"] = const.tile([P, D], F32, tag="b3bc", bufs=4, name=f"bvbc_{p}")

    # long-lived activations
    qT = {}
    res_nat = {}
    pts = {}
    for p in ("a", "b"):
        qT[p] = const.tile([P, KT, NS], BF16, tag="qT", bufs=2, name=f"qT_{p}")
        res_nat[p] = const.tile([P, IT, D], F32, tag="resnat", bufs=2, name=f"resnat_{p}")
        pts[p] = const.tile([P, NT, NS], BF16, tag="pts", bufs=2, name=f"pts_{p}")

    # DRAM bounce buffers for collectives
    kag_in = {p: dram.tile([D, NS], BF16, tag=f"kag_in_{p}", name=f"kag_in_{p}") for p in ("a", "b")}
    kag_out = {p: dram.tile([R, D, NS], BF16, tag=f"kag_out_{p}", name=f"kag_out_{p}", addr_space="Shared") for p in ("a", "b")}
    vag_in = {p: dram.tile([NS, D], BF16, tag=f"vag_in_{p}", name=f"vag_in_{p}") for p in ("a", "b")}
    vag_out = {p: dram.tile([R, NS, D], BF16, tag=f"vag_out_{p}", name=f"vag_out_{p}", addr_space="Shared") for p in ("a", "b")}
    cs_in = {p: dram.tile([1, N], F32, tag=f"cs_in_{p}", name=f"cs_in_{p}") for p in ("a", "b")}
    cs_out = {p: dram.tile([1, N], F32, tag=f"cs_out_{p}", name=f"cs_out_{p}", addr_space="Shared") for p in ("a", "b")}

    # ================= stage 1+2: projections, K/V (+gathers), Q, residual =================
    with tc.tile_pool(name="lin", bufs=1) as lin, \
         tc.tile_pool(name="ps12", bufs=1, space="PSUM") as ps12:

        def load_w(wname):
            # per-k-chunk DMAs: matmuls can start on the first 256KB instead of
            # waiting for the full 1MB, and the stream interleaves more finely
            # with collective traffic
            w = lin.tile([P, KT, D], F32R, tag="w", bufs=3, name=f"w_{wname}")
            src = ins[wname].rearrange("(k p) o -> p k o", p=P).bitcast(F32R)
            for k in range(KT):
                nc.sync.dma_start(w[:, k, :], src[:, k, :])
            return w

        def linT(x, wname, bias_t=None, func=AF.Copy, out_dtype=F32R, tag="act", bufs=4, name=None, out=None):
            """Feature-major linear: y^T = func(W @ x^T + b); x, y: [128, KT, NS]."""
            w = load_w(wname)
            y = out if out is not None else lin.tile([P, KT, NS], out_dtype, tag=tag, bufs=bufs, name=name or f"y_{wname}")
            for j in range(KT):
                pm = ps12.tile([P, NS], F32, tag="lin", bufs=4, name="pm_lin")
                for k in range(KT):
                    nc.tensor.matmul(pm[:], w[:, k, j * P:(j + 1) * P], x[:, k, :],
                                     start=(k == 0), stop=(k == KT - 1))
                bias_ap = bias_t[:, j:j + 1] if bias_t is not None else 0.0
                nc.scalar.activation(y[:, j, :], pm[:], AF.Identity if func == AF.Copy and bias_t is not None else func,
                                     bias=bias_ap)
            return y

        def linN(x, wname, out_nat, bias_bcast=None):
            """Row-major linear: out_nat[:, it, :] = x^T_it @ W^T (+b); x feature-major f32r,
            out_nat [128, IT, D] (dtype from tile)."""
            w = load_w(wname)
            for it in range(IT):
                pm = ps12.tile([P, D], F32, tag="nat", bufs=2, name="pm_nat")
                for k in range(KT):
                    nc.tensor.matmul(pm[:], x[:, k, it * P:(it + 1) * P], w[:, k, :],
                                     start=(k == 0), stop=(k == KT - 1))
                if bias_bcast is not None:
                    nc.vector.tensor_add(pm[:], pm[:], bias_bcast[:])
                nc.scalar.activation(out_nat[:, it, :], pm[:], AF.Copy)

        # K/V/Q weights are pre-folded through pW2 on the host, so they consume
        # the first-layer activations h directly — the K gathers launch one
        # matmul-layer earlier. Branch b first so the K_b gather leads.
        hact = {}
        for p in ("b", "a"):
            x0 = lin.tile([P, KT, NS], F32R, tag="fin", bufs=2, name=f"fin_{p}")
            x0src = ins["faT" if p == "a" else "fbT"].rearrange("(k p) i -> p k i", p=P).bitcast(F32R)
            for k in range(KT):
                nc.sync.dma_start(x0[:, k, :], x0src[:, k, :])
            hact[p] = linT(x0, f"{p}_pw1", bias[p, "pb1"], AF.Relu, tag="h", bufs=3, name=f"h_{p}")
            kT = linT(hact[p], f"{p}_wk", bias[p, "bk"], AF.Identity, out_dtype=BF16, tag="kv", bufs=2, name=f"kT_{p}")
            for k in range(KT):
                nc.sync.dma_start(kag_in[p][k * P:(k + 1) * P, :], kT[:, k, :])
            nc.gpsimd.collective_compute("AllGather", ALU.bypass, replica_groups=RG,
                                         ins=[kag_in[p].opt()], outs=[kag_out[p].opt()])

        # deferred constants via the scalar (Activation) DMA queue, off the
        # startup-critical path but BEFORE their first consumers (bvbc feeds
        # the V projection below)
        for p in ("a", "b"):
            nc.scalar.dma_start(bias[p, "bvbc"][:], ins[f"{p}_bvbc"][:, :])
            nc.scalar.dma_start(bias[p, "rb3bc"][:], ins[f"{p}_rb3bc"][:, :])
            for g in ("lng", "lnb"):
                nc.scalar.dma_start(lnp[p, g][:], ins[f"{p}_{g}"][:, :])

        # V in natural layout (bf16), gathered in need-order (V_a for PV_a first)
        for p in ("a", "b"):
            v_nat = lin.tile([P, IT, D], BF16, tag="vnat", bufs=2, name=f"vnat_{p}")
            linN(hact[p], f"{p}_wv", v_nat, bias_bcast=bias[p, "bvbc"])
            for it in range(IT):
                nc.sync.dma_start(vag_in[p][it * P:(it + 1) * P, :], v_nat[:, it, :])
            nc.gpsimd.collective_compute("AllGather", ALU.bypass, replica_groups=RG,
                                         ins=[vag_in[p].opt()], outs=[vag_out[p].opt()])

        # local compute that overlaps the gathers
        for p in ("a", "b"):
            linT(hact[p], f"{p}_wq", bias[p, "bq"], AF.Identity, out=qT[p])
        feat = {}
        for p in ("a", "b"):
            feat[p] = linT(hact[p], f"{p}_pw2", bias[p, "pb2"], AF.Identity, tag="feat", bufs=2, name=f"feat_{p}")
        for p in ("a", "b"):
            r1 = linT(feat[p], f"{p}_rw1", bias[p, "rb1"], AF.Relu, tag="h", bufs=3)
            r2 = linT(r1, f"{p}_rw2", bias[p, "rb2"], AF.Relu, tag="h", bufs=3)
            linN(r2, f"{p}_rw3", res_nat[p], bias_bcast=bias[p, "rb3bc"])

    # ================= attention scores + online softmax + P^T, per branch =================
    rcs = {}
    for p in ("a", "b"):
        rcs[p] = const.tile([P, NT], F32, tag="rcs", bufs=2, name=f"rcs_{p}")

    def attn_scores(p, o, mid_hook=None):
        with tc.tile_pool(name=f"s_{p}", bufs=1) as sp, \
             tc.tile_pool(name=f"psS_{p}", bufs=1, space="PSUM") as ps:
            # bf16 storage of exp'd scores is safe: values <= 1 after the
            # running-max subtraction; logits are consumed straight from PSUM f32.
            sgb = [sp.tile([P, N], BF16, tag="sg", bufs=IT, name=f"sg_{p}_{it}") for it in range(IT)]
            m_hist = [sp.tile([P, R], F32, tag="mh", bufs=IT, name=f"mh_{p}_{it}") for it in range(IT)]
            negm_h = [sp.tile([P, R], F32, tag="nh", bufs=IT, name=f"nh_{p}_{it}") for it in range(IT)]
            s_hist = [sp.tile([P, R], F32, tag="sh", bufs=IT, name=f"sh_{p}_{it}") for it in range(IT)]
            csf = sp.tile([1, N], F32, tag="csf", bufs=1, name=f"csf_{p}")
            for r in range(R):
                kch = sp.tile([P, KT, NS], BF16, tag="kch", bufs=R, name=f"kch_{p}")
                nc.sync.dma_start(kch[:], kag_out[o][r, :, :].rearrange("(k p) i -> p k i", p=P))
                for it in range(IT):
                    pm = ps.tile([P, NS], F32, tag="s", bufs=3, name="pm_s")
                    for k in range(KT):
                        nc.tensor.matmul(pm[:], qT[p][:, k, it * P:(it + 1) * P], kch[:, k, :],
                                         start=(k == 0), stop=(k == KT - 1))
                    if r == 0:
                        nc.vector.tensor_reduce(m_hist[it][:, 0:1], pm[:], AX.X, ALU.max)
                    else:
                        mxc = sp.tile([P, 1], F32, tag="mxc", bufs=4, name="mxc")
                        nc.vector.tensor_reduce(mxc[:], pm[:], AX.X, ALU.max)
                        nc.vector.tensor_tensor(m_hist[it][:, r:r + 1], m_hist[it][:, r - 1:r], mxc[:], ALU.max)
                    nc.vector.tensor_scalar_mul(negm_h[it][:, r:r + 1], m_hist[it][:, r:r + 1], -1.0)
                    # exp straight out of PSUM (f32 logits), bf16 store, chunk sum accumulated
                    nc.scalar.activation(sgb[it][:, r * NS:(r + 1) * NS], pm[:], AF.Exp,
                                         bias=negm_h[it][:, r:r + 1], accum_out=s_hist[it][:, r:r + 1])
            # correction factors exp(m_r - m_final); rowsum = sum_r s_r * fac_r.
            # Batched per engine to minimize cross-engine round trips.
            fac = [sp.tile([P, R], F32, tag="fac", bufs=IT, name=f"fac_{it}") for it in range(IT)]
            sf = [sp.tile([P, R], F32, tag="sf", bufs=IT, name=f"sf_{it}") for it in range(IT)]
            rlf = [sp.tile([P, 1], F32, tag="rlf", bufs=IT, name=f"rlf_{it}") for it in range(IT)]
            for it in range(IT):
                nc.scalar.activation(fac[it][:], m_hist[it][:], AF.Exp, bias=negm_h[it][:, R - 1:R])
            for it in range(IT):
                nc.vector.tensor_tensor(sf[it][:], s_hist[it][:], fac[it][:], ALU.mult)
                nc.vector.tensor_reduce(rlf[it][:], sf[it][:], AX.X, ALU.add)
                nc.vector.reciprocal(rlf[it][:], rlf[it][:])
            for it in range(IT):
                nc.scalar.activation(fac[it][:], fac[it][:], AF.Copy, scale=rlf[it][:])
            if mid_hook is not None:
                mid_hook()
            # normalization scales r-major (split scalar/vector), each chunk's
            # column-sum matmuls issued right behind its scales so the AllReduce
            # input is complete ~when the scale pass ends
            for r in range(R):
                for it in range(IT):
                    sl = sgb[it][:, r * NS:(r + 1) * NS]
                    if it % 2 == 0:
                        nc.scalar.activation(sl, sl, AF.Copy, scale=fac[it][:, r:r + 1])
                    else:
                        nc.vector.tensor_scalar_mul(sl, sl, fac[it][:, r:r + 1])
                pc = ps.tile([1, NS], F32, tag="col", bufs=2, name="pm_col")
                for it in range(IT):
                    nc.tensor.matmul(pc[:], onesb[:], sgb[it][:, r * NS:(r + 1) * NS],
                                     start=(it == 0), stop=(it == IT - 1))
                nc.scalar.activation(csf[:, r * NS:(r + 1) * NS], pc[:], AF.Copy)
            # P^T tiles: 4 transposes share one PSUM bank -> one strided copy
            for it in range(IT):
                for tq in range(NT // 4):
                    pm = ps.tile([P, 4, P], BF16, tag="tp", bufs=2, name="pm_tp")
                    for tt in range(4):
                        nc.tensor.transpose(pm[:, tt, :], sgb[it][:, (4 * tq + tt) * P:(4 * tq + tt + 1) * P],
                                            identb[:])
                    if tq % 2 == 0:
                        nc.vector.tensor_copy(pts[p][:, 4 * tq:4 * tq + 4, it * P:(it + 1) * P], pm[:])
                    else:
                        nc.scalar.activation(pts[p][:, 4 * tq:4 * tq + 4, it * P:(it + 1) * P], pm[:], AF.Copy)
            # contiguous 16KB column-sum payload on the (idle) gpsimd queue
            nc.gpsimd.dma_start(cs_in[p][:, :], csf[:, :])
            nc.gpsimd.collective_compute("AllReduce", ALU.add, replica_groups=RG,
                                         ins=[cs_in[p].opt()], outs=[cs_out[p].opt()])
            nc.gpsimd.dma_start(rcs[p][:], cs_out[p][:, :].rearrange("o (t p) -> p (o t)", p=P))

    def _recip_hook(p):
        # 1/(eps + colsum): tiny vector ops slotted early in the other branch's
        # vector stream, right after that branch's fac math
        def h():
            nc.vector.tensor_scalar_add(rcs[p][:], rcs[p][:], EPS_ATTN)
            nc.vector.reciprocal(rcs[p][:], rcs[p][:])
        return h

    attn_scores("a", "b")
    attn_scores("b", "a", mid_hook=_recip_hook("a"))

    # ================= PV + epilogue, branches interleaved =================
    with tc.tile_pool(name="tail", bufs=1) as tail, \
         tc.tile_pool(name="psB", bufs=1, space="PSUM") as psB:

        ffnw = {}
        vchs = {}
        for p in ("a", "b"):
            ffnw[p] = tail.tile([P, KT, D], BF16, tag="ffnw", bufs=2, name=f"ffnw_{p}")
            nc.sync.dma_start(ffnw[p][:], ins[f"{p}_ffn"].rearrange("(k p) o -> p k o", p=P))
        # prefetch ALL V chunks for both branches (deps: the V all-gathers only)
        for p in ("a", "b"):
            vchs[p] = [tail.tile([P, IT, D], BF16, tag="vch", bufs=11, name=f"vch_{p}_{r}")
                       for r in range(R)]
            for r in range(R):
                nc.sync.dma_start(vchs[p][r][:], vag_out[p][r, :, :].rearrange("(j p) d -> p j d", p=P))

        def pv_prep(p):
            """V-chunk scaling by rcs, split across scalar and vector queues
            (reciprocal for branch a was already slotted into branch b's
            attention; do branch b's here)."""
            if p == "b":
                nc.vector.tensor_scalar_add(rcs[p][:], rcs[p][:], EPS_ATTN)
                nc.vector.reciprocal(rcs[p][:], rcs[p][:])
            for r in range(R):
                for jj in range(IT):
                    t = IT * r + jj
                    if jj % 2 == 0:
                        nc.scalar.activation(vchs[p][r][:, jj, :], vchs[p][r][:, jj, :],
                                             AF.Copy, scale=rcs[p][:, t:t + 1])
                    else:
                        nc.vector.tensor_scalar_mul(vchs[p][r][:, jj, :], vchs[p][r][:, jj, :],
                                                    rcs[p][:, t:t + 1])

        def pv_mm(p):
            # it-major: each output tile's accumulation completes early, so the
            # LN -> transpose -> FFN chain pipelines behind the remaining matmuls
            pvs = [psB.tile([P, D], F32, tag="pv", bufs=IT, name=f"pm_pv_{p}_{it}") for it in range(IT)]
            for it in range(IT):
                for r in range(R):
                    vch = vchs[p][r]
                    for jj in range(IT):
                        t = IT * r + jj
                        nc.tensor.matmul(pvs[it][:], pts[p][:, t, it * P:(it + 1) * P], vch[:, jj, :],
                                         start=(r == 0 and jj == 0), stop=(r == R - 1 and jj == IT - 1))
            return pvs

        def epilogue_pre(p, pvs):
            """PSUM drain + add residual + LN -> y (f32) and yb (bf16)."""
            ys, ybs = [], []
            for it in range(IT):
                x = tail.tile([P, D], F32, tag="x", bufs=4, name=f"x_{p}_{it}")
                nc.vector.tensor_add(x[:], pvs[it][:], res_nat[p][:, it, :])
                negmu = tail.tile([P, 1], F32, tag="negmu", bufs=4, name=f"negmu_{it}")
                nc.vector.tensor_reduce(negmu[:], x[:], AX.X, ALU.add, negate=True)
                nc.scalar.mul(negmu[:], negmu[:], 1.0 / D)
                xc = tail.tile([P, D], F32, tag="xc", bufs=2, name=f"xc_{it}")
                nc.scalar.add(xc[:], x[:], negmu[:])
                scr = tail.tile([P, D], F32, tag="x", bufs=4, name=f"scr_{it}")
                ssq = tail.tile([P, 1], F32, tag="ssq", bufs=4, name=f"ssq_{it}")
                nc.scalar.activation(scr[:], xc[:], AF.Square, accum_out=ssq[:])
                std = tail.tile([P, 1], F32, tag="std", bufs=4, name=f"std_{it}")
                nc.vector.tensor_scalar(std[:], ssq[:], 1.0 / D, EPS_LN, ALU.mult, ALU.add)
                nc.scalar.sqrt(std[:], std[:])
                rstd = tail.tile([P, 1], F32, tag="rstd", bufs=4, name=f"rstd_{it}")
                nc.vector.reciprocal(rstd[:], std[:])
                y = tail.tile([P, D], F32, tag="y", bufs=2 * IT, name=f"y_{p}_{it}")
                nc.vector.scalar_tensor_tensor(y[:], xc[:], rstd[:], lnp[p, "lng"][:],
                                               op0=ALU.mult, op1=ALU.mult)
                nc.vector.tensor_add(y[:], y[:], lnp[p, "lnb"][:])
                yb = tail.tile([P, D], BF16, tag="yb", bufs=2 * IT, name=f"yb_{p}_{it}")
                nc.vector.tensor_copy(yb[:], y[:])
                ys.append(y)
                ybs.append(yb)
            return ys, ybs

        def epilogue_ffn(p, ys, ybs):
            bi = 0 if p == "a" else 1
            asT = tail.tile([P, KT, NS], BF16, tag="asT", bufs=2, name=f"asT_{p}")
            for it in range(IT):
                for k in range(KT):
                    pm = psB.tile([P, P], BF16, tag="tp", bufs=2, name="pm_tpb")
                    nc.tensor.transpose(pm[:], ybs[it][:, k * P:(k + 1) * P], identb[:])
                    nc.vector.tensor_copy(asT[:, k, it * P:(it + 1) * P], pm[:])
            for it in range(IT):
                pf = psB.tile([P, D], F32, tag="ffn", bufs=2, name="pm_ffn")
                for k in range(KT):
                    nc.tensor.matmul(pf[:], asT[:, k, it * P:(it + 1) * P], ffnw[p][:, k, :],
                                     start=(k == 0), stop=(k == KT - 1))
                outt = tail.tile([P, D], F32, tag="outt", bufs=2, name=f"outt_{it}")
                nc.vector.tensor_add(outt[:], ys[it][:], pf[:])
                nc.sync.dma_start(out_ext[bi, it * P:(it + 1) * P, :], outt[:])

        pv_prep("a")
        pvs_a = pv_mm("a")
        ys_a, ybs_a = epilogue_pre("a", pvs_a)   # frees pvs_a banks for pv_b
        pv_prep("b")
        pvs_b = pv_mm("b")                        # tensor works here while LN_a runs
        epilogue_ffn("a", ys_a, ybs_a)
        ys_b, ybs_b = epilogue_pre("b", pvs_b)
        epilogue_ffn("b", ys_b, ybs_b)

    stack.close()


_CACHE = {}


def _build():
    if "nc" in _CACHE:
        return _CACHE["nc"]
    nc = bacc.Bacc("TRN2", target_bir_lowering=False, debug=False, num_devices=R)
    ins = {}
    for nm, shape in (("faT", [D, NS]), ("fbT", [D, NS])):
        ins[nm] = nc.dram_tensor(nm, shape, F32, kind="ExternalInput")
    for p in ("a", "b"):
        for w in WNAMES:
            ins[f"{p}_{w}"] = nc.dram_tensor(f"{p}_{w}", [D, D], F32, kind="ExternalInput")
        ins[f"{p}_ffn"] = nc.dram_tensor(f"{p}_ffn", [D, D], BF16, kind="ExternalInput")
        for b in BNAMES + ("bk", "bq"):
            ins[f"{p}_{b}"] = nc.dram_tensor(f"{p}_{b}", [P, KT], F32, kind="ExternalInput")
        ins[f"{p}_rb3bc"] = nc.dram_tensor(f"{p}_rb3bc", [P, D], F32, kind="ExternalInput")
        ins[f"{p}_bvbc"] = nc.dram_tensor(f"{p}_bvbc", [P, D], F32, kind="ExternalInput")
        for g in ("lng", "lnb"):
            ins[f"{p}_{g}"] = nc.dram_tensor(f"{p}_{g}", [P, D], F32, kind="ExternalInput")
    out_ext = nc.dram_tensor("out", [2, NS, D], F32, kind="ExternalOutput")

    with tile.TileContext(nc) as tc:
        _body(tc, {k: v.ap() for k, v in ins.items()}, out_ext.ap())
    nc.compile()
    _CACHE["nc"] = nc
    return nc


def _r32r(x):
    xi = np.ascontiguousarray(x, dtype=np.float32).view(np.uint32)
    xi = (xi + np.uint32(1 << 12)) & np.uint32(0xFFFFE000)
    return xi.view(np.float32)


_WMAP = {"pw1": "pW1", "pw2": "pW2", "rw1": "rW1", "rw2": "rW2", "rw3": "rW3",
         "wq": "Wq", "wk": "Wk", "wv": "Wv"}
_BMAP = {"pb1": "pb1", "pb2": "pb2", "rb1": "rb1", "rb2": "rb2"}


def _prep_in_maps(inputs):
    shared = {}
    for p, pre in (("A", "a"), ("B", "b")):
        # fold the second projection layer into the Q/K/V weights (host-side):
        # K = feat@Wk.T = h@(Wk@pW2).T + (Wk@pb2)
        pW2 = np.asarray(inputs[f"{p}_pW2"], dtype=np.float64)
        pb2 = np.asarray(inputs[f"{p}_pb2"], dtype=np.float64)
        for dn, rn in (("wk", "Wk"), ("wq", "Wq"), ("wv", "Wv")):
            Wx = np.asarray(inputs[f"{p}_{rn}"], dtype=np.float64)
            inputs = dict(inputs)
            inputs[f"{p}_{rn}_folded"] = (Wx @ pW2).astype(np.float32)
            inputs[f"{p}_{rn}_bias"] = (Wx @ pb2).astype(np.float32)
        for dn, rn in _WMAP.items():
            src = f"{p}_{rn}_folded" if dn in ("wk", "wq", "wv") else f"{p}_{rn}"
            shared[f"{pre}_{dn}"] = _r32r(np.ascontiguousarray(np.asarray(inputs[src]).T.astype(np.float32)))
        shared[f"{pre}_bk"] = np.ascontiguousarray(
            np.asarray(inputs[f"{p}_Wk_bias"]).reshape(KT, P).T)
        shared[f"{pre}_bq"] = np.ascontiguousarray(
            np.asarray(inputs[f"{p}_Wq_bias"]).reshape(KT, P).T)
        shared[f"{pre}_bvbc"] = np.ascontiguousarray(
            np.broadcast_to(np.asarray(inputs[f"{p}_Wv_bias"]), (P, D)).astype(np.float32))
        shared[f"{pre}_ffn"] = np.ascontiguousarray(
            np.asarray(inputs[f"{p}_ffnW"]).T.astype(np.float32)).astype(ml_dtypes.bfloat16)
        for dn, rn in _BMAP.items():
            shared[f"{pre}_{dn}"] = np.ascontiguousarray(
                np.asarray(inputs[f"{p}_{rn}"]).astype(np.float32).reshape(KT, P).T)
        shared[f"{pre}_rb3bc"] = np.ascontiguousarray(
            np.broadcast_to(np.asarray(inputs[f"{p}_rb3"]).astype(np.float32), (P, D)))
        shared[f"{pre}_lng"] = np.ascontiguousarray(
            np.broadcast_to(np.asarray(inputs[f"{p}_ln_g"]).astype(np.float32), (P, D)))
        shared[f"{pre}_lnb"] = np.ascontiguousarray(
            np.broadcast_to(np.asarray(inputs[f"{p}_ln_b"]).astype(np.float32), (P, D)))
    fa = np.asarray(inputs["feat_a"]).astype(np.float32)
    fb = np.asarray(inputs["feat_b"]).astype(np.float32)
    in_maps = []
    for r in range(R):
        m = dict(shared)
        m["faT"] = _r32r(np.ascontiguousarray(fa[r * NS:(r + 1) * NS].T))
        m["fbT"] = _r32r(np.ascontiguousarray(fb[r * NS:(r + 1) * NS].T))
        in_maps.append(m)
    return in_maps


def kernel(**inputs) -> np.ndarray:
    nc = _build()
    in_maps = _prep_in_maps(inputs)
    res = bass_utils.run_bass_kernel_spmd(nc, in_maps, core_ids=list(range(R)))
    outs = res.results
    A = np.concatenate([outs[r]["out"][0] for r in range(R)], axis=0)
    B = np.concatenate([outs[r]["out"][1] for r in range(R)], axis=0)
    return np.stack([A, B], axis=0)


# revision 50
# speedup vs baseline: 1.1892x; 1.1892x over previous
"""Trainium2 Bass kernel for the dual-branch cross-attention block (N=4096, D=512).

Sharding: rows of Q / output across 8 cores (512 rows each). K and V shards are
computed locally and all-gathered; the column-sum renorm uses a tiny AllReduce.

v2 layout/schedule:
- linears in f32r; V / P-transpose storage / FFN path in bf16 (rel_err ~5e-3).
- V and the residual-MLP output are produced directly in natural (row-major)
  layout by using the activation tiles as the matmul stationary operand.
- Row softmax scale (1/rowsum) is applied to the exp'd scores on the scalar
  engine before the P transposes; column sums then come from cheap vector
  reduces over the transposed tiles (no tensor-engine colsum matmuls).
- Branches are interleaved (S_a, S_b, PV_a+epilogue_a, PV_b+epilogue_b) so both
  column-sum AllReduces hide under tensor work; V gathers are bf16.
"""
import numpy as np

try:
    import concourse.bass as bass  # noqa: F401
except ImportError:
    import sys
    sys.path.insert(0, "/opt/trn_rl_repo")
import concourse.bass as bass
import concourse.tile as tile
from concourse import bacc, mybir
from concourse import bass_utils
from concourse.masks import make_identity
import ml_dtypes

F32 = mybir.dt.float32
F32R = mybir.dt.float32r
BF16 = mybir.dt.bfloat16
AF = mybir.ActivationFunctionType
ALU = mybir.AluOpType
AX = mybir.AxisListType

N, D, R, NS, P = 4096, 512, 8, 512, 128
KT = D // P   # 4 d-tiles
IT = NS // P  # 4 i-tiles per core
NT = N // P   # 32 n-tiles global
RG = [list(range(R))]
EPS_LN, EPS_ATTN = 1e-6, 1e-9

WNAMES = ("pw1", "rw1", "rw2", "rw3", "wq", "wk", "wv")
BNAMES = ("pb1", "pb2", "rb1", "rb2")


def _body(tc, ins, out_ext):
    nc = tc.nc

    import contextlib
    stack = contextlib.ExitStack()

    # ---- whole-kernel pools ----
    const = stack.enter_context(tc.tile_pool(name="const", bufs=1))
    dram = stack.enter_context(tc.tile_pool(name="dram", bufs=1, space="DRAM"))

    ident = const.tile([P, P], F32, tag="ident", bufs=1)
    make_identity(nc, ident[:])
    identr = const.tile([P, P], F32R, tag="identr", bufs=1)
    nc.vector.tensor_copy(identr[:], ident[:])
    identb = const.tile([P, P], BF16, tag="identb", bufs=1)
    nc.vector.tensor_copy(identb[:], ident[:])
    onesb = const.tile([P, 1], BF16, tag="onesb", bufs=1)
    nc.vector.memset(onesb[:], 1.0)
    epsln = const.tile([P, 1], F32, tag="epsln", bufs=1)
    nc.vector.memset(epsln[:], EPS_LN)

    # LN params (pre-replicated on host to [128, 512]); DMAs deferred past the
    # startup-critical x0/weight loads (lnp only needed in the epilogue).
    lnp = {}
    for p in ("a", "b"):
        for g in ("lng", "lnb"):
            lnp[p, g] = const.tile([P, D], F32, tag="lnp", bufs=4, name=f"lnp_{p}_{g}")
    # feature-major biases [128, 4]; rb3 broadcast [128, 512]
    bias = {}
    for p in ("a", "b"):
        for b in BNAMES + ("bk", "bq"):
            t = const.tile([P, KT], F32, tag="bias", bufs=12, name=f"bias_{p}_{b}")
            nc.sync.dma_start(t[:], ins[f"{p}_{b}"][:, :])
            bias[p, b] = t
        bias[p, "rb3bc"] = const.tile([P, D], F32, tag="b3bc", bufs=4, name=f"b3bc_{p}")
        bias[p, "bvbc"] = const.tile([P, D], F32, tag="b3bc", bufs=4, name=f"bvbc_{p}")

    # long-lived activations
    qT = {}
    res_nat = {}
    pts = {}
    for p in ("a", "b"):
        qT[p] = const.tile([P, KT, NS], BF16, tag="qT", bufs=2, name=f"qT_{p}")
        res_nat[p] = const.tile([P, IT, D], F32, tag="resnat", bufs=2, name=f"resnat_{p}")
        pts[p] = const.tile([P, NT, NS], BF16, tag="pts", bufs=2, name=f"pts_{p}")

    # DRAM bounce buffers for collectives
    kag_in = {p: dram.tile([D, NS], BF16, tag=f"kag_in_{p}", name=f"kag_in_{p}") for p in ("a", "b")}
    kag_out = {p: dram.tile([R, D, NS], BF16, tag=f"kag_out_{p}", name=f"kag_out_{p}", addr_space="Shared") for p in ("a", "b")}
    vag_in = {p: dram.tile([NS, D], BF16, tag=f"vag_in_{p}", name=f"vag_in_{p}") for p in ("a", "b")}
    vag_out = {p: dram.tile([R, NS, D], BF16, tag=f"vag_out_{p}", name=f"vag_out_{p}", addr_space="Shared") for p in ("a", "b")}
    cs_in = {p: dram.tile([1, N], F32, tag=f"cs_in_{p}", name=f"cs_in_{p}") for p in ("a", "b")}
    cs_out = {p: dram.tile([1, N], F32, tag=f"cs_out_{p}", name=f"cs_out_{p}", addr_space="Shared") for p in ("a", "b")}

    # ================= stage 1+2: projections, K/V (+gathers), Q, residual =================
    with tc.tile_pool(name="lin", bufs=1) as lin, \
         tc.tile_pool(name="ps12", bufs=1, space="PSUM") as ps12:

        def load_w(wname):
            # per-k-chunk DMAs: matmuls can start on the first 256KB instead of
            # waiting for the full 1MB, and the stream interleaves more finely
            # with collective traffic
            w = lin.tile([P, KT, D], F32R, tag="w", bufs=3, name=f"w_{wname}")
            src = ins[wname].rearrange("(k p) o -> p k o", p=P).bitcast(F32R)
            for k in range(KT):
                nc.sync.dma_start(w[:, k, :], src[:, k, :])
            return w

        def linT(x, wname, bias_t=None, func=AF.Copy, out_dtype=F32R, tag="act", bufs=4, name=None, out=None):
            """Feature-major linear: y^T = func(W @ x^T + b); x, y: [128, KT, NS]."""
            w = load_w(wname)
            y = out if out is not None else lin.tile([P, KT, NS], out_dtype, tag=tag, bufs=bufs, name=name or f"y_{wname}")
            for j in range(KT):
                pm = ps12.tile([P, NS], F32, tag="lin", bufs=4, name="pm_lin")
                for k in range(KT):
                    nc.tensor.matmul(pm[:], w[:, k, j * P:(j + 1) * P], x[:, k, :],
                                     start=(k == 0), stop=(k == KT - 1))
                bias_ap = bias_t[:, j:j + 1] if bias_t is not None else 0.0
                nc.scalar.activation(y[:, j, :], pm[:], AF.Identity if func == AF.Copy and bias_t is not None else func,
                                     bias=bias_ap)
            return y

        def linN(x, wname, out_nat, bias_bcast=None):
            """Row-major linear: out_nat[:, it, :] = x^T_it @ W^T (+b); x feature-major f32r,
            out_nat [128, IT, D] (dtype from tile)."""
            w = load_w(wname)
            for it in range(IT):
                pm = ps12.tile([P, D], F32, tag="nat", bufs=2, name="pm_nat")
                for k in range(KT):
                    nc.tensor.matmul(pm[:], x[:, k, it * P:(it + 1) * P], w[:, k, :],
                                     start=(k == 0), stop=(k == KT - 1))
                if bias_bcast is not None:
                    nc.vector.tensor_add(pm[:], pm[:], bias_bcast[:])
                nc.scalar.activation(out_nat[:, it, :], pm[:], AF.Copy)

        # K/V/Q weights are pre-folded through pW2 on the host, so they consume
        # the first-layer activations h directly — the K gathers launch one
        # matmul-layer earlier. Branch b first so the K_b gather leads.
        hact = {}
        for p in ("b", "a"):
            x0 = lin.tile([P, KT, NS], F32R, tag="fin", bufs=2, name=f"fin_{p}")
            x0src = ins["faT" if p == "a" else "fbT"].rearrange("(k p) i -> p k i", p=P).bitcast(F32R)
            for k in range(KT):
                nc.sync.dma_start(x0[:, k, :], x0src[:, k, :])
            hact[p] = linT(x0, f"{p}_pw1", bias[p, "pb1"], AF.Relu, tag="h", bufs=3, name=f"h_{p}")
            kT = linT(hact[p], f"{p}_wk", bias[p, "bk"], AF.Identity, out_dtype=BF16, tag="kv", bufs=2, name=f"kT_{p}")
            for k in range(KT):
                nc.sync.dma_start(kag_in[p][k * P:(k + 1) * P, :], kT[:, k, :])
            nc.gpsimd.collective_compute("AllGather", ALU.bypass, replica_groups=RG,
                                         ins=[kag_in[p].opt()], outs=[kag_out[p].opt()])

        # deferred constants via the scalar (Activation) DMA queue, off the
        # startup-critical path but BEFORE their first consumers (bvbc feeds
        # the V projection below)
        for p in ("a", "b"):
            nc.scalar.dma_start(bias[p, "bvbc"][:], ins[f"{p}_bvbc"][:, :])
            nc.scalar.dma_start(bias[p, "rb3bc"][:], ins[f"{p}_rb3bc"][:, :])
            for g in ("lng", "lnb"):
                nc.scalar.dma_start(lnp[p, g][:], ins[f"{p}_{g}"][:, :])

        # V in natural layout (bf16), gathered in need-order (V_a for PV_a first)
        for p in ("a", "b"):
            v_nat = lin.tile([P, IT, D], BF16, tag="vnat", bufs=2, name=f"vnat_{p}")
            linN(hact[p], f"{p}_wv", v_nat, bias_bcast=bias[p, "bvbc"])
            for it in range(IT):
                nc.sync.dma_start(vag_in[p][it * P:(it + 1) * P, :], v_nat[:, it, :])
            nc.gpsimd.collective_compute("AllGather", ALU.bypass, replica_groups=RG,
                                         ins=[vag_in[p].opt()], outs=[vag_out[p].opt()])

        # local compute that overlaps the gathers; rw1 is also pre-folded
        # through pW2 on the host, so feat is never materialized
        for p in ("a", "b"):
            linT(hact[p], f"{p}_wq", bias[p, "bq"], AF.Identity, out=qT[p])
        for p in ("a", "b"):
            r1 = linT(hact[p], f"{p}_rw1", bias[p, "rb1"], AF.Relu, tag="h", bufs=3)
            r2 = linT(r1, f"{p}_rw2", bias[p, "rb2"], AF.Relu, tag="h", bufs=3)
            linN(r2, f"{p}_rw3", res_nat[p], bias_bcast=bias[p, "rb3bc"])

    # ================= attention scores + online softmax + P^T, per branch =================
    rcs = {}
    for p in ("a", "b"):
        rcs[p] = const.tile([P, NT], F32, tag="rcs", bufs=2, name=f"rcs_{p}")

    def attn_scores(p, o, mid_hook=None):
        with tc.tile_pool(name=f"s_{p}", bufs=1) as sp, \
             tc.tile_pool(name=f"psS_{p}", bufs=1, space="PSUM") as ps:
            # bf16 storage of exp'd scores is safe: values <= 1 after the
            # running-max subtraction; logits are consumed straight from PSUM f32.
            sgb = [sp.tile([P, N], BF16, tag="sg", bufs=IT, name=f"sg_{p}_{it}") for it in range(IT)]
            m_hist = [sp.tile([P, R], F32, tag="mh", bufs=IT, name=f"mh_{p}_{it}") for it in range(IT)]
            negm_h = [sp.tile([P, R], F32, tag="nh", bufs=IT, name=f"nh_{p}_{it}") for it in range(IT)]
            s_hist = [sp.tile([P, R], F32, tag="sh", bufs=IT, name=f"sh_{p}_{it}") for it in range(IT)]
            csf = sp.tile([1, N], F32, tag="csf", bufs=1, name=f"csf_{p}")
            for r in range(R):
                kch = sp.tile([P, KT, NS], BF16, tag="kch", bufs=R, name=f"kch_{p}")
                nc.sync.dma_start(kch[:], kag_out[o][r, :, :].rearrange("(k p) i -> p k i", p=P))
                for it in range(IT):
                    pm = ps.tile([P, NS], F32, tag="s", bufs=3, name="pm_s")
                    for k in range(KT):
                        nc.tensor.matmul(pm[:], qT[p][:, k, it * P:(it + 1) * P], kch[:, k, :],
                                         start=(k == 0), stop=(k == KT - 1))
                    if r == 0:
                        nc.vector.tensor_reduce(m_hist[it][:, 0:1], pm[:], AX.X, ALU.max)
                    else:
                        mxc = sp.tile([P, 1], F32, tag="mxc", bufs=4, name="mxc")
                        nc.vector.tensor_reduce(mxc[:], pm[:], AX.X, ALU.max)
                        nc.vector.tensor_tensor(m_hist[it][:, r:r + 1], m_hist[it][:, r - 1:r], mxc[:], ALU.max)
                    nc.vector.tensor_scalar_mul(negm_h[it][:, r:r + 1], m_hist[it][:, r:r + 1], -1.0)
                    # exp straight out of PSUM (f32 logits), bf16 store, chunk sum accumulated
                    nc.scalar.activation(sgb[it][:, r * NS:(r + 1) * NS], pm[:], AF.Exp,
                                         bias=negm_h[it][:, r:r + 1], accum_out=s_hist[it][:, r:r + 1])
            # correction factors exp(m_r - m_final); rowsum = sum_r s_r * fac_r.
            # Batched per engine to minimize cross-engine round trips.
            fac = [sp.tile([P, R], F32, tag="fac", bufs=IT, name=f"fac_{it}") for it in range(IT)]
            sf = [sp.tile([P, R], F32, tag="sf", bufs=IT, name=f"sf_{it}") for it in range(IT)]
            rlf = [sp.tile([P, 1], F32, tag="rlf", bufs=IT, name=f"rlf_{it}") for it in range(IT)]
            for it in range(IT):
                nc.scalar.activation(fac[it][:], m_hist[it][:], AF.Exp, bias=negm_h[it][:, R - 1:R])
            for it in range(IT):
                nc.vector.tensor_tensor(sf[it][:], s_hist[it][:], fac[it][:], ALU.mult)
                nc.vector.tensor_reduce(rlf[it][:], sf[it][:], AX.X, ALU.add)
                nc.vector.reciprocal(rlf[it][:], rlf[it][:])
            for it in range(IT):
                nc.scalar.activation(fac[it][:], fac[it][:], AF.Copy, scale=rlf[it][:])
            if mid_hook is not None:
                mid_hook()
            # normalization scales r-major (split scalar/vector), each chunk's
            # column-sum matmuls issued right behind its scales so the AllReduce
            # input is complete ~when the scale pass ends
            for r in range(R):
                for it in range(IT):
                    sl = sgb[it][:, r * NS:(r + 1) * NS]
                    if it % 2 == 0:
                        nc.scalar.activation(sl, sl, AF.Copy, scale=fac[it][:, r:r + 1])
                    else:
                        nc.vector.tensor_scalar_mul(sl, sl, fac[it][:, r:r + 1])
                pc = ps.tile([1, NS], F32, tag="col", bufs=2, name="pm_col")
                for it in range(IT):
                    nc.tensor.matmul(pc[:], onesb[:], sgb[it][:, r * NS:(r + 1) * NS],
                                     start=(it == 0), stop=(it == IT - 1))
                nc.scalar.activation(csf[:, r * NS:(r + 1) * NS], pc[:], AF.Copy)
            # P^T tiles: 4 transposes share one PSUM bank -> one strided copy
            for it in range(IT):
                for tq in range(NT // 4):
                    pm = ps.tile([P, 4, P], BF16, tag="tp", bufs=2, name="pm_tp")
                    for tt in range(4):
                        nc.tensor.transpose(pm[:, tt, :], sgb[it][:, (4 * tq + tt) * P:(4 * tq + tt + 1) * P],
                                            identb[:])
                    if tq % 2 == 0:
                        nc.vector.tensor_copy(pts[p][:, 4 * tq:4 * tq + 4, it * P:(it + 1) * P], pm[:])
                    else:
                        nc.scalar.activation(pts[p][:, 4 * tq:4 * tq + 4, it * P:(it + 1) * P], pm[:], AF.Copy)
            # contiguous 16KB column-sum payload on the (idle) gpsimd queue
            nc.gpsimd.dma_start(cs_in[p][:, :], csf[:, :])
            nc.gpsimd.collective_compute("AllReduce", ALU.add, replica_groups=RG,
                                         ins=[cs_in[p].opt()], outs=[cs_out[p].opt()])
            nc.gpsimd.dma_start(rcs[p][:], cs_out[p][:, :].rearrange("o (t p) -> p (o t)", p=P))

    def _recip_hook(p):
        # 1/(eps + colsum): tiny vector ops slotted early in the other branch's
        # vector stream, right after that branch's fac math
        def h():
            nc.vector.tensor_scalar_add(rcs[p][:], rcs[p][:], EPS_ATTN)
            nc.vector.reciprocal(rcs[p][:], rcs[p][:])
        return h

    attn_scores("a", "b")
    attn_scores("b", "a", mid_hook=_recip_hook("a"))

    # ================= PV + epilogue, branches interleaved =================
    with tc.tile_pool(name="tail", bufs=1) as tail, \
         tc.tile_pool(name="psB", bufs=1, space="PSUM") as psB:

        ffnw = {}
        vchs = {}
        for p in ("a", "b"):
            ffnw[p] = tail.tile([P, KT, D], BF16, tag="ffnw", bufs=2, name=f"ffnw_{p}")
            nc.sync.dma_start(ffnw[p][:], ins[f"{p}_ffn"].rearrange("(k p) o -> p k o", p=P))
        # prefetch ALL V chunks for both branches (deps: the V all-gathers only)
        for p in ("a", "b"):
            vchs[p] = [tail.tile([P, IT, D], BF16, tag="vch", bufs=11, name=f"vch_{p}_{r}")
                       for r in range(R)]
            for r in range(R):
                nc.sync.dma_start(vchs[p][r][:], vag_out[p][r, :, :].rearrange("(j p) d -> p j d", p=P))

        def pv_prep(p):
            """V-chunk scaling by rcs, split across scalar and vector queues
            (reciprocal for branch a was already slotted into branch b's
            attention; do branch b's here)."""
            if p == "b":
                nc.vector.tensor_scalar_add(rcs[p][:], rcs[p][:], EPS_ATTN)
                nc.vector.reciprocal(rcs[p][:], rcs[p][:])
            for r in range(R):
                for jj in range(IT):
                    t = IT * r + jj
                    if jj % 2 == 0:
                        nc.scalar.activation(vchs[p][r][:, jj, :], vchs[p][r][:, jj, :],
                                             AF.Copy, scale=rcs[p][:, t:t + 1])
                    else:
                        nc.vector.tensor_scalar_mul(vchs[p][r][:, jj, :], vchs[p][r][:, jj, :],
                                                    rcs[p][:, t:t + 1])

        def pv_mm(p):
            # it-major: each output tile's accumulation completes early, so the
            # LN -> transpose -> FFN chain pipelines behind the remaining matmuls
            pvs = [psB.tile([P, D], F32, tag="pv", bufs=IT, name=f"pm_pv_{p}_{it}") for it in range(IT)]
            for it in range(IT):
                for r in range(R):
                    vch = vchs[p][r]
                    for jj in range(IT):
                        t = IT * r + jj
                        nc.tensor.matmul(pvs[it][:], pts[p][:, t, it * P:(it + 1) * P], vch[:, jj, :],
                                         start=(r == 0 and jj == 0), stop=(r == R - 1 and jj == IT - 1))
            return pvs

        def epilogue_pre(p, pvs):
            """PSUM drain + add residual + LN -> y (f32) and yb (bf16)."""
            ys, ybs = [], []
            for it in range(IT):
                x = tail.tile([P, D], F32, tag="x", bufs=4, name=f"x_{p}_{it}")
                nc.vector.tensor_add(x[:], pvs[it][:], res_nat[p][:, it, :])
                negmu = tail.tile([P, 1], F32, tag="negmu", bufs=4, name=f"negmu_{it}")
                nc.vector.tensor_reduce(negmu[:], x[:], AX.X, ALU.add, negate=True)
                nc.scalar.mul(negmu[:], negmu[:], 1.0 / D)
                xc = tail.tile([P, D], F32, tag="xc", bufs=2, name=f"xc_{it}")
                nc.scalar.add(xc[:], x[:], negmu[:])
                scr = tail.tile([P, D], F32, tag="x", bufs=4, name=f"scr_{it}")
                ssq = tail.tile([P, 1], F32, tag="ssq", bufs=4, name=f"ssq_{it}")
                nc.scalar.activation(scr[:], xc[:], AF.Square, accum_out=ssq[:])
                std = tail.tile([P, 1], F32, tag="std", bufs=4, name=f"std_{it}")
                nc.vector.tensor_scalar(std[:], ssq[:], 1.0 / D, EPS_LN, ALU.mult, ALU.add)
                nc.scalar.sqrt(std[:], std[:])
                rstd = tail.tile([P, 1], F32, tag="rstd", bufs=4, name=f"rstd_{it}")
                nc.vector.reciprocal(rstd[:], std[:])
                y = tail.tile([P, D], F32, tag="y", bufs=2 * IT, name=f"y_{p}_{it}")
                nc.vector.scalar_tensor_tensor(y[:], xc[:], rstd[:], lnp[p, "lng"][:],
                                               op0=ALU.mult, op1=ALU.mult)
                nc.vector.tensor_add(y[:], y[:], lnp[p, "lnb"][:])
                yb = tail.tile([P, D], BF16, tag="yb", bufs=2 * IT, name=f"yb_{p}_{it}")
                nc.vector.tensor_copy(yb[:], y[:])
                ys.append(y)
                ybs.append(yb)
            return ys, ybs

        def epilogue_ffn(p, ys, ybs):
            bi = 0 if p == "a" else 1
            asT = tail.tile([P, KT, NS], BF16, tag="asT", bufs=2, name=f"asT_{p}")
            for it in range(IT):
                for k in range(KT):
                    pm = psB.tile([P, P], BF16, tag="tp", bufs=2, name="pm_tpb")
                    nc.tensor.transpose(pm[:], ybs[it][:, k * P:(k + 1) * P], identb[:])
                    nc.vector.tensor_copy(asT[:, k, it * P:(it + 1) * P], pm[:])
            for it in range(IT):
                pf = psB.tile([P, D], F32, tag="ffn", bufs=2, name="pm_ffn")
                for k in range(KT):
                    nc.tensor.matmul(pf[:], asT[:, k, it * P:(it + 1) * P], ffnw[p][:, k, :],
                                     start=(k == 0), stop=(k == KT - 1))
                outt = tail.tile([P, D], F32, tag="outt", bufs=2, name=f"outt_{it}")
                nc.vector.tensor_add(outt[:], ys[it][:], pf[:])
                nc.sync.dma_start(out_ext[bi, it * P:(it + 1) * P, :], outt[:])

        pv_prep("a")
        pvs_a = pv_mm("a")
        ys_a, ybs_a = epilogue_pre("a", pvs_a)   # frees pvs_a banks for pv_b
        pv_prep("b")
        pvs_b = pv_mm("b")                        # tensor works here while LN_a runs
        epilogue_ffn("a", ys_a, ybs_a)
        ys_b, ybs_b = epilogue_pre("b", pvs_b)
        epilogue_ffn("b", ys_b, ybs_b)

    stack.close()


_CACHE = {}


def _build():
    if "nc" in _CACHE:
        return _CACHE["nc"]
    nc = bacc.Bacc("TRN2", target_bir_lowering=False, debug=False, num_devices=R)
    ins = {}
    for nm, shape in (("faT", [D, NS]), ("fbT", [D, NS])):
        ins[nm] = nc.dram_tensor(nm, shape, F32, kind="ExternalInput")
    for p in ("a", "b"):
        for w in WNAMES:
            ins[f"{p}_{w}"] = nc.dram_tensor(f"{p}_{w}", [D, D], F32, kind="ExternalInput")
        ins[f"{p}_ffn"] = nc.dram_tensor(f"{p}_ffn", [D, D], BF16, kind="ExternalInput")
        for b in BNAMES + ("bk", "bq"):
            ins[f"{p}_{b}"] = nc.dram_tensor(f"{p}_{b}", [P, KT], F32, kind="ExternalInput")
        ins[f"{p}_rb3bc"] = nc.dram_tensor(f"{p}_rb3bc", [P, D], F32, kind="ExternalInput")
        ins[f"{p}_bvbc"] = nc.dram_tensor(f"{p}_bvbc", [P, D], F32, kind="ExternalInput")
        for g in ("lng", "lnb"):
            ins[f"{p}_{g}"] = nc.dram_tensor(f"{p}_{g}", [P, D], F32, kind="ExternalInput")
    out_ext = nc.dram_tensor("out", [2, NS, D], F32, kind="ExternalOutput")

    with tile.TileContext(nc) as tc:
        _body(tc, {k: v.ap() for k, v in ins.items()}, out_ext.ap())
    nc.compile()
    _CACHE["nc"] = nc
    return nc


def _r32r(x):
    xi = np.ascontiguousarray(x, dtype=np.float32).view(np.uint32)
    xi = (xi + np.uint32(1 << 12)) & np.uint32(0xFFFFE000)
    return xi.view(np.float32)


_WMAP = {"pw1": "pW1", "rw1": "rW1", "rw2": "rW2", "rw3": "rW3",
         "wq": "Wq", "wk": "Wk", "wv": "Wv"}
_BMAP = {"pb1": "pb1", "pb2": "pb2", "rb1": "rb1", "rb2": "rb2"}


def _prep_in_maps(inputs):
    shared = {}
    for p, pre in (("A", "a"), ("B", "b")):
        # fold the second projection layer into the Q/K/V weights (host-side):
        # K = feat@Wk.T = h@(Wk@pW2).T + (Wk@pb2)
        pW2 = np.asarray(inputs[f"{p}_pW2"], dtype=np.float64)
        pb2 = np.asarray(inputs[f"{p}_pb2"], dtype=np.float64)
        inputs = dict(inputs)
        for dn, rn in (("wk", "Wk"), ("wq", "Wq"), ("wv", "Wv"), ("rw1", "rW1")):
            Wx = np.asarray(inputs[f"{p}_{rn}"], dtype=np.float64)
            inputs[f"{p}_{rn}_folded"] = (Wx @ pW2).astype(np.float32)
            inputs[f"{p}_{rn}_bias"] = (Wx @ pb2).astype(np.float32)
        # rw1's folded bias merges with its own bias
        inputs[f"{p}_rb1"] = (np.asarray(inputs[f"{p}_rW1_bias"], dtype=np.float64)
                              + np.asarray(inputs[f"{p}_rb1"], dtype=np.float64)).astype(np.float32)
        for dn, rn in _WMAP.items():
            src = f"{p}_{rn}_folded" if dn in ("wk", "wq", "wv", "rw1") else f"{p}_{rn}"
            shared[f"{pre}_{dn}"] = _r32r(np.ascontiguousarray(np.asarray(inputs[src]).T.astype(np.float32)))
        shared[f"{pre}_bk"] = np.ascontiguousarray(
            np.asarray(inputs[f"{p}_Wk_bias"]).reshape(KT, P).T)
        shared[f"{pre}_bq"] = np.ascontiguousarray(
            np.asarray(inputs[f"{p}_Wq_bias"]).reshape(KT, P).T)
        shared[f"{pre}_bvbc"] = np.ascontiguousarray(
            np.broadcast_to(np.asarray(inputs[f"{p}_Wv_bias"]), (P, D)).astype(np.float32))
        shared[f"{pre}_ffn"] = np.ascontiguousarray(
            np.asarray(inputs[f"{p}_ffnW"]).T.astype(np.float32)).astype(ml_dtypes.bfloat16)
        for dn, rn in _BMAP.items():
            shared[f"{pre}_{dn}"] = np.ascontiguousarray(
                np.asarray(inputs[f"{p}_{rn}"]).astype(np.float32).reshape(KT, P).T)
        shared[f"{pre}_rb3bc"] = np.ascontiguousarray(
            np.broadcast_to(np.asarray(inputs[f"{p}_rb3"]).astype(np.float32), (P, D)))
        shared[f"{pre}_lng"] = np.ascontiguousarray(
            np.broadcast_to(np.asarray(inputs[f"{p}_ln_g"]).astype(np.float32), (P, D)))
        shared[f"{pre}_lnb"] = np.ascontiguousarray(
            np.broadcast_to(np.asarray(inputs[f"{p}_ln_b"]).astype(np.float32), (P, D)))
    fa = np.asarray(inputs["feat_a"]).astype(np.float32)
    fb = np.asarray(inputs["feat_b"]).astype(np.float32)
    in_maps = []
    for r in range(R):
        m = dict(shared)
        m["faT"] = _r32r(np.ascontiguousarray(fa[r * NS:(r + 1) * NS].T))
        m["fbT"] = _r32r(np.ascontiguousarray(fb[r * NS:(r + 1) * NS].T))
        in_maps.append(m)
    return in_maps


def kernel(**inputs) -> np.ndarray:
    nc = _build()
    in_maps = _prep_in_maps(inputs)
    res = bass_utils.run_bass_kernel_spmd(nc, in_maps, core_ids=list(range(R)))
    outs = res.results
    A = np.concatenate([outs[r]["out"][0] for r in range(R)], axis=0)
    B = np.concatenate([outs[r]["out"][1] for r in range(R)], axis=0)
    return np.stack([A, B], axis=0)


# revision 52
# speedup vs baseline: 1.2117x; 1.0189x over previous
"""Trainium2 Bass kernel for the dual-branch cross-attention block (N=4096, D=512).

Sharding: rows of Q / output across 8 cores (512 rows each). K and V shards are
computed locally and all-gathered; the column-sum renorm uses a tiny AllReduce.

v2 layout/schedule:
- linears in f32r; V / P-transpose storage / FFN path in bf16 (rel_err ~5e-3).
- V and the residual-MLP output are produced directly in natural (row-major)
  layout by using the activation tiles as the matmul stationary operand.
- Row softmax scale (1/rowsum) is applied to the exp'd scores on the scalar
  engine before the P transposes; column sums then come from cheap vector
  reduces over the transposed tiles (no tensor-engine colsum matmuls).
- Branches are interleaved (S_a, S_b, PV_a+epilogue_a, PV_b+epilogue_b) so both
  column-sum AllReduces hide under tensor work; V gathers are bf16.
"""
import numpy as np

try:
    import concourse.bass as bass  # noqa: F401
except ImportError:
    import sys
    sys.path.insert(0, "/opt/trn_rl_repo")
import concourse.bass as bass
import concourse.tile as tile
from concourse import bacc, mybir
from concourse import bass_utils
from concourse.masks import make_identity
import ml_dtypes

F32 = mybir.dt.float32
F32R = mybir.dt.float32r
BF16 = mybir.dt.bfloat16
AF = mybir.ActivationFunctionType
ALU = mybir.AluOpType
AX = mybir.AxisListType

N, D, R, NS, P = 4096, 512, 8, 512, 128
KT = D // P   # 4 d-tiles
IT = NS // P  # 4 i-tiles per core
NT = N // P   # 32 n-tiles global
RG = [list(range(R))]
EPS_LN, EPS_ATTN = 1e-6, 1e-9

WNAMES = ("pw1", "rw1", "rw2", "rw3", "wq", "wk", "wv")
BNAMES = ("pb1", "pb2", "rb1", "rb2")


def _body(tc, ins, out_ext):
    nc = tc.nc

    import contextlib
    stack = contextlib.ExitStack()

    # ---- whole-kernel pools ----
    const = stack.enter_context(tc.tile_pool(name="const", bufs=1))
    dram = stack.enter_context(tc.tile_pool(name="dram", bufs=1, space="DRAM"))

    ident = const.tile([P, P], F32, tag="ident", bufs=1)
    make_identity(nc, ident[:])
    identr = const.tile([P, P], F32R, tag="identr", bufs=1)
    nc.vector.tensor_copy(identr[:], ident[:])
    identb = const.tile([P, P], BF16, tag="identb", bufs=1)
    nc.vector.tensor_copy(identb[:], ident[:])
    onesb = const.tile([P, 1], BF16, tag="onesb", bufs=1)
    nc.vector.memset(onesb[:], 1.0)
    epsln = const.tile([P, 1], F32, tag="epsln", bufs=1)
    nc.vector.memset(epsln[:], EPS_LN)

    # LN params (pre-replicated on host to [128, 512]); DMAs deferred past the
    # startup-critical x0/weight loads (lnp only needed in the epilogue).
    lnp = {}
    for p in ("a", "b"):
        for g in ("lng", "lnb"):
            lnp[p, g] = const.tile([P, D], F32, tag="lnp", bufs=4, name=f"lnp_{p}_{g}")
    # feature-major biases [128, 4]; rb3 broadcast [128, 512]
    bias = {}
    for p in ("a", "b"):
        for b in BNAMES + ("bk", "bq"):
            t = const.tile([P, KT], F32, tag="bias", bufs=12, name=f"bias_{p}_{b}")
            nc.sync.dma_start(t[:], ins[f"{p}_{b}"][:, :])
            bias[p, b] = t
        bias[p, "rb3bc"] = const.tile([P, D], F32, tag="b3bc", bufs=4, name=f"b3bc_{p}")
        bias[p, "bvbc"] = const.tile([P, D], F32, tag="b3bc", bufs=4, name=f"bvbc_{p}")

    # long-lived activations
    qT = {}
    res_nat = {}
    pts = {}
    for p in ("a", "b"):
        qT[p] = const.tile([P, KT, NS], BF16, tag="qT", bufs=2, name=f"qT_{p}")
        res_nat[p] = const.tile([P, IT, D], F32, tag="resnat", bufs=2, name=f"resnat_{p}")
        pts[p] = const.tile([P, NT, NS], BF16, tag="pts", bufs=2, name=f"pts_{p}")

    # DRAM bounce buffers for collectives
    kag_in = {p: dram.tile([D, NS], BF16, tag=f"kag_in_{p}", name=f"kag_in_{p}") for p in ("a", "b")}
    kag_out = {p: dram.tile([R, D, NS], BF16, tag=f"kag_out_{p}", name=f"kag_out_{p}", addr_space="Shared") for p in ("a", "b")}
    vag_in = {p: dram.tile([NS, D], BF16, tag=f"vag_in_{p}", name=f"vag_in_{p}") for p in ("a", "b")}
    vag_out = {p: dram.tile([R, NS, D], BF16, tag=f"vag_out_{p}", name=f"vag_out_{p}", addr_space="Shared") for p in ("a", "b")}
    cs_in = {p: dram.tile([1, N], F32, tag=f"cs_in_{p}", name=f"cs_in_{p}") for p in ("a", "b")}
    cs_out = {p: dram.tile([1, N], F32, tag=f"cs_out_{p}", name=f"cs_out_{p}", addr_space="Shared") for p in ("a", "b")}

    # ================= stage 1+2: projections, K/V (+gathers), Q, residual =================
    with tc.tile_pool(name="lin", bufs=1) as lin, \
         tc.tile_pool(name="ps12", bufs=1, space="PSUM") as ps12:

        def load_w(wname):
            # per-k-chunk DMAs: matmuls can start on the first 256KB instead of
            # waiting for the full 1MB, and the stream interleaves more finely
            # with collective traffic
            w = lin.tile([P, KT, D], F32R, tag="w", bufs=3, name=f"w_{wname}")
            src = ins[wname].rearrange("(k p) o -> p k o", p=P).bitcast(F32R)
            for k in range(KT):
                nc.sync.dma_start(w[:, k, :], src[:, k, :])
            return w

        def linT(x, wname, bias_t=None, func=AF.Copy, out_dtype=F32R, tag="act", bufs=4, name=None, out=None):
            """Feature-major linear: y^T = func(W @ x^T + b); x, y: [128, KT, NS]."""
            w = load_w(wname)
            y = out if out is not None else lin.tile([P, KT, NS], out_dtype, tag=tag, bufs=bufs, name=name or f"y_{wname}")
            for j in range(KT):
                pm = ps12.tile([P, NS], F32, tag="lin", bufs=4, name="pm_lin")
                for k in range(KT):
                    nc.tensor.matmul(pm[:], w[:, k, j * P:(j + 1) * P], x[:, k, :],
                                     start=(k == 0), stop=(k == KT - 1))
                bias_ap = bias_t[:, j:j + 1] if bias_t is not None else 0.0
                nc.scalar.activation(y[:, j, :], pm[:], AF.Identity if func == AF.Copy and bias_t is not None else func,
                                     bias=bias_ap)
            return y

        def linN(x, wname, out_nat, bias_bcast=None):
            """Row-major linear: out_nat[:, it, :] = x^T_it @ W^T (+b); x feature-major f32r,
            out_nat [128, IT, D] (dtype from tile)."""
            w = load_w(wname)
            for it in range(IT):
                pm = ps12.tile([P, D], F32, tag="nat", bufs=2, name="pm_nat")
                for k in range(KT):
                    nc.tensor.matmul(pm[:], x[:, k, it * P:(it + 1) * P], w[:, k, :],
                                     start=(k == 0), stop=(k == KT - 1))
                if bias_bcast is not None:
                    nc.vector.tensor_add(pm[:], pm[:], bias_bcast[:])
                nc.scalar.activation(out_nat[:, it, :], pm[:], AF.Copy)

        # K/V/Q weights are pre-folded through pW2 on the host, so they consume
        # the first-layer activations h directly — the K gathers launch one
        # matmul-layer earlier. Branch b first so the K_b gather leads.
        hact = {}
        for p in ("b", "a"):
            x0 = lin.tile([P, KT, NS], F32R, tag="fin", bufs=2, name=f"fin_{p}")
            x0src = ins["faT" if p == "a" else "fbT"].rearrange("(k p) i -> p k i", p=P).bitcast(F32R)
            for k in range(KT):
                nc.sync.dma_start(x0[:, k, :], x0src[:, k, :])
            hact[p] = linT(x0, f"{p}_pw1", bias[p, "pb1"], AF.Relu, tag="h", bufs=3, name=f"h_{p}")
            kT = linT(hact[p], f"{p}_wk", bias[p, "bk"], AF.Identity, out_dtype=BF16, tag="kv", bufs=2, name=f"kT_{p}")
            for k in range(KT):
                nc.sync.dma_start(kag_in[p][k * P:(k + 1) * P, :], kT[:, k, :])
            nc.gpsimd.collective_compute("AllGather", ALU.bypass, replica_groups=RG,
                                         ins=[kag_in[p].opt()], outs=[kag_out[p].opt()])

        # deferred constants via the scalar (Activation) DMA queue, off the
        # startup-critical path but BEFORE their first consumers (bvbc feeds
        # the V projection below)
        for p in ("a", "b"):
            nc.scalar.dma_start(bias[p, "bvbc"][:], ins[f"{p}_bvbc"][:, :])
            nc.scalar.dma_start(bias[p, "rb3bc"][:], ins[f"{p}_rb3bc"][:, :])
            for g in ("lng", "lnb"):
                nc.scalar.dma_start(lnp[p, g][:], ins[f"{p}_{g}"][:, :])

        # V in natural layout (bf16), gathered in need-order (V_a for PV_a first)
        for p in ("a", "b"):
            v_nat = lin.tile([P, IT, D], BF16, tag="vnat", bufs=2, name=f"vnat_{p}")
            linN(hact[p], f"{p}_wv", v_nat, bias_bcast=bias[p, "bvbc"])
            for it in range(IT):
                nc.sync.dma_start(vag_in[p][it * P:(it + 1) * P, :], v_nat[:, it, :])
            nc.gpsimd.collective_compute("AllGather", ALU.bypass, replica_groups=RG,
                                         ins=[vag_in[p].opt()], outs=[vag_out[p].opt()])

        # local compute that overlaps the gathers; rw1 is also pre-folded
        # through pW2 on the host, so feat is never materialized
        for p in ("a", "b"):
            linT(hact[p], f"{p}_wq", bias[p, "bq"], AF.Identity, out=qT[p])
        for p in ("a", "b"):
            r1 = linT(hact[p], f"{p}_rw1", bias[p, "rb1"], AF.Relu, tag="h", bufs=3)
            r2 = linT(r1, f"{p}_rw2", bias[p, "rb2"], AF.Relu, tag="h", bufs=3)
            linN(r2, f"{p}_rw3", res_nat[p], bias_bcast=bias[p, "rb3bc"])

    # ================= attention scores + online softmax + P^T, per branch =================
    rcs = {}
    for p in ("a", "b"):
        rcs[p] = const.tile([P, NT], F32, tag="rcs", bufs=2, name=f"rcs_{p}")

    def attn_scores(p, o, mid_hook=None):
        with tc.tile_pool(name=f"s_{p}", bufs=1) as sp, \
             tc.tile_pool(name=f"psS_{p}", bufs=1, space="PSUM") as ps:
            # bf16 storage of exp'd scores is safe: values <= 1 after the
            # running-max subtraction; logits are consumed straight from PSUM f32.
            sgb = [sp.tile([P, N], BF16, tag="sg", bufs=IT, name=f"sg_{p}_{it}") for it in range(IT)]
            m_hist = [sp.tile([P, R], F32, tag="mh", bufs=IT, name=f"mh_{p}_{it}") for it in range(IT)]
            negm_h = [sp.tile([P, R], F32, tag="nh", bufs=IT, name=f"nh_{p}_{it}") for it in range(IT)]
            s_hist = [sp.tile([P, R], F32, tag="sh", bufs=IT, name=f"sh_{p}_{it}") for it in range(IT)]
            csf = sp.tile([1, N], F32, tag="csf", bufs=1, name=f"csf_{p}")
            for r in range(R):
                kch = sp.tile([P, KT, NS], BF16, tag="kch", bufs=R, name=f"kch_{p}")
                nc.sync.dma_start(kch[:], kag_out[o][r, :, :].rearrange("(k p) i -> p k i", p=P))
                for it in range(IT):
                    pm = ps.tile([P, NS], F32, tag="s", bufs=4, name="pm_s")
                    for k in range(KT):
                        nc.tensor.matmul(pm[:], qT[p][:, k, it * P:(it + 1) * P], kch[:, k, :],
                                         start=(k == 0), stop=(k == KT - 1))
                    if r == 0:
                        nc.vector.tensor_reduce(m_hist[it][:, 0:1], pm[:], AX.X, ALU.max)
                    else:
                        mxc = sp.tile([P, 1], F32, tag="mxc", bufs=4, name="mxc")
                        nc.vector.tensor_reduce(mxc[:], pm[:], AX.X, ALU.max)
                        nc.vector.tensor_tensor(m_hist[it][:, r:r + 1], m_hist[it][:, r - 1:r], mxc[:], ALU.max)
                    nc.vector.tensor_scalar_mul(negm_h[it][:, r:r + 1], m_hist[it][:, r:r + 1], -1.0)
                    # exp straight out of PSUM (f32 logits), bf16 store, chunk sum accumulated
                    nc.scalar.activation(sgb[it][:, r * NS:(r + 1) * NS], pm[:], AF.Exp,
                                         bias=negm_h[it][:, r:r + 1], accum_out=s_hist[it][:, r:r + 1])
            # correction factors exp(m_r - m_final); rowsum = sum_r s_r * fac_r.
            # Batched per engine to minimize cross-engine round trips.
            fac = [sp.tile([P, R], F32, tag="fac", bufs=IT, name=f"fac_{it}") for it in range(IT)]
            sf = [sp.tile([P, R], F32, tag="sf", bufs=IT, name=f"sf_{it}") for it in range(IT)]
            rlf = [sp.tile([P, 1], F32, tag="rlf", bufs=IT, name=f"rlf_{it}") for it in range(IT)]
            for it in range(IT):
                nc.scalar.activation(fac[it][:], m_hist[it][:], AF.Exp, bias=negm_h[it][:, R - 1:R])
            for it in range(IT):
                nc.vector.tensor_tensor(sf[it][:], s_hist[it][:], fac[it][:], ALU.mult)
                nc.vector.tensor_reduce(rlf[it][:], sf[it][:], AX.X, ALU.add)
                nc.vector.reciprocal(rlf[it][:], rlf[it][:])
            for it in range(IT):
                nc.scalar.activation(fac[it][:], fac[it][:], AF.Copy, scale=rlf[it][:])
            if mid_hook is not None:
                mid_hook()
            # normalization scales r-major (split scalar/vector), each chunk's
            # column-sum matmuls issued right behind its scales so the AllReduce
            # input is complete ~when the scale pass ends
            for r in range(R):
                for it in range(IT):
                    sl = sgb[it][:, r * NS:(r + 1) * NS]
                    if it % 2 == 0:
                        nc.scalar.activation(sl, sl, AF.Copy, scale=fac[it][:, r:r + 1])
                    else:
                        nc.vector.tensor_scalar_mul(sl, sl, fac[it][:, r:r + 1])
                pc = ps.tile([1, NS], F32, tag="col", bufs=2, name="pm_col")
                for it in range(IT):
                    nc.tensor.matmul(pc[:], onesb[:], sgb[it][:, r * NS:(r + 1) * NS],
                                     start=(it == 0), stop=(it == IT - 1))
                nc.scalar.activation(csf[:, r * NS:(r + 1) * NS], pc[:], AF.Copy)
            # P^T tiles: 4 transposes share one PSUM bank -> one strided copy
            for it in range(IT):
                for tq in range(NT // 4):
                    pm = ps.tile([P, 4, P], BF16, tag="tp", bufs=2, name="pm_tp")
                    for tt in range(4):
                        nc.tensor.transpose(pm[:, tt, :], sgb[it][:, (4 * tq + tt) * P:(4 * tq + tt + 1) * P],
                                            identb[:])
                    if tq % 2 == 0:
                        nc.vector.tensor_copy(pts[p][:, 4 * tq:4 * tq + 4, it * P:(it + 1) * P], pm[:])
                    else:
                        nc.scalar.activation(pts[p][:, 4 * tq:4 * tq + 4, it * P:(it + 1) * P], pm[:], AF.Copy)
            # contiguous 16KB column-sum payload on the (idle) gpsimd queue
            nc.gpsimd.dma_start(cs_in[p][:, :], csf[:, :])
            nc.gpsimd.collective_compute("AllReduce", ALU.add, replica_groups=RG,
                                         ins=[cs_in[p].opt()], outs=[cs_out[p].opt()])
            nc.gpsimd.dma_start(rcs[p][:], cs_out[p][:, :].rearrange("o (t p) -> p (o t)", p=P))

    def _recip_hook(p):
        # 1/(eps + colsum): tiny vector ops slotted early in the other branch's
        # vector stream, right after that branch's fac math
        def h():
            nc.vector.tensor_scalar_add(rcs[p][:], rcs[p][:], EPS_ATTN)
            nc.vector.reciprocal(rcs[p][:], rcs[p][:])
        return h

    attn_scores("a", "b")
    attn_scores("b", "a", mid_hook=_recip_hook("a"))

    # ================= PV + epilogue, branches interleaved =================
    with tc.tile_pool(name="tail", bufs=1) as tail, \
         tc.tile_pool(name="psB", bufs=1, space="PSUM") as psB:

        ffnw = {}
        vchs = {}
        for p in ("a", "b"):
            ffnw[p] = tail.tile([P, KT, D], BF16, tag="ffnw", bufs=2, name=f"ffnw_{p}")
            nc.sync.dma_start(ffnw[p][:], ins[f"{p}_ffn"].rearrange("(k p) o -> p k o", p=P))
        # prefetch ALL V chunks for both branches (deps: the V all-gathers only)
        for p in ("a", "b"):
            vchs[p] = [tail.tile([P, IT, D], BF16, tag="vch", bufs=11, name=f"vch_{p}_{r}")
                       for r in range(R)]
            for r in range(R):
                nc.sync.dma_start(vchs[p][r][:], vag_out[p][r, :, :].rearrange("(j p) d -> p j d", p=P))

        def pv_prep(p):
            """V-chunk scaling by rcs, split across scalar and vector queues
            (reciprocal for branch a was already slotted into branch b's
            attention; do branch b's here)."""
            if p == "b":
                nc.vector.tensor_scalar_add(rcs[p][:], rcs[p][:], EPS_ATTN)
                nc.vector.reciprocal(rcs[p][:], rcs[p][:])
            for r in range(R):
                for jj in range(IT):
                    t = IT * r + jj
                    if jj % 2 == 0:
                        nc.scalar.activation(vchs[p][r][:, jj, :], vchs[p][r][:, jj, :],
                                             AF.Copy, scale=rcs[p][:, t:t + 1])
                    else:
                        nc.vector.tensor_scalar_mul(vchs[p][r][:, jj, :], vchs[p][r][:, jj, :],
                                                    rcs[p][:, t:t + 1])

        def pv_mm(p):
            # it-major: each output tile's accumulation completes early, so the
            # LN -> transpose -> FFN chain pipelines behind the remaining matmuls
            pvs = [psB.tile([P, D], F32, tag="pv", bufs=IT, name=f"pm_pv_{p}_{it}") for it in range(IT)]
            for it in range(IT):
                for r in range(R):
                    vch = vchs[p][r]
                    for jj in range(IT):
                        t = IT * r + jj
                        nc.tensor.matmul(pvs[it][:], pts[p][:, t, it * P:(it + 1) * P], vch[:, jj, :],
                                         start=(r == 0 and jj == 0), stop=(r == R - 1 and jj == IT - 1))
            return pvs

        def epilogue_pre(p, pvs):
            """PSUM drain + add residual + LN -> y (f32) and yb (bf16)."""
            ys, ybs = [], []
            for it in range(IT):
                x = tail.tile([P, D], F32, tag="x", bufs=4, name=f"x_{p}_{it}")
                nc.vector.tensor_add(x[:], pvs[it][:], res_nat[p][:, it, :])
                negmu = tail.tile([P, 1], F32, tag="negmu", bufs=4, name=f"negmu_{it}")
                nc.vector.tensor_reduce(negmu[:], x[:], AX.X, ALU.add, negate=True)
                nc.scalar.mul(negmu[:], negmu[:], 1.0 / D)
                xc = tail.tile([P, D], F32, tag="xc", bufs=2, name=f"xc_{it}")
                nc.scalar.add(xc[:], x[:], negmu[:])
                scr = tail.tile([P, D], F32, tag="x", bufs=4, name=f"scr_{it}")
                ssq = tail.tile([P, 1], F32, tag="ssq", bufs=4, name=f"ssq_{it}")
                nc.scalar.activation(scr[:], xc[:], AF.Square, accum_out=ssq[:])
                std = tail.tile([P, 1], F32, tag="std", bufs=4, name=f"std_{it}")
                nc.vector.tensor_scalar(std[:], ssq[:], 1.0 / D, EPS_LN, ALU.mult, ALU.add)
                nc.scalar.sqrt(std[:], std[:])
                rstd = tail.tile([P, 1], F32, tag="rstd", bufs=4, name=f"rstd_{it}")
                nc.vector.reciprocal(rstd[:], std[:])
                y = tail.tile([P, D], F32, tag="y", bufs=2 * IT, name=f"y_{p}_{it}")
                nc.vector.scalar_tensor_tensor(y[:], xc[:], rstd[:], lnp[p, "lng"][:],
                                               op0=ALU.mult, op1=ALU.mult)
                nc.vector.tensor_add(y[:], y[:], lnp[p, "lnb"][:])
                yb = tail.tile([P, D], BF16, tag="yb", bufs=2 * IT, name=f"yb_{p}_{it}")
                nc.vector.tensor_copy(yb[:], y[:])
                ys.append(y)
                ybs.append(yb)
            return ys, ybs

        def epilogue_ffn(p, ys, ybs):
            bi = 0 if p == "a" else 1
            asT = tail.tile([P, KT, NS], BF16, tag="asT", bufs=2, name=f"asT_{p}")
            for it in range(IT):
                pm = psB.tile([P, KT, P], BF16, tag="tp", bufs=2, name="pm_tpb")
                for k in range(KT):
                    nc.tensor.transpose(pm[:, k, :], ybs[it][:, k * P:(k + 1) * P], identb[:])
                nc.vector.tensor_copy(asT[:, 0:KT, it * P:(it + 1) * P], pm[:])
            for it in range(IT):
                pf = psB.tile([P, D], F32, tag="ffn", bufs=2, name="pm_ffn")
                for k in range(KT):
                    nc.tensor.matmul(pf[:], asT[:, k, it * P:(it + 1) * P], ffnw[p][:, k, :],
                                     start=(k == 0), stop=(k == KT - 1))
                outt = tail.tile([P, D], F32, tag="outt", bufs=2, name=f"outt_{it}")
                nc.vector.tensor_add(outt[:], ys[it][:], pf[:])
                nc.sync.dma_start(out_ext[bi, it * P:(it + 1) * P, :], outt[:])

        pv_prep("a")
        pvs_a = pv_mm("a")
        ys_a, ybs_a = epilogue_pre("a", pvs_a)   # frees pvs_a banks for pv_b
        pv_prep("b")
        pvs_b = pv_mm("b")                        # tensor works here while LN_a runs
        epilogue_ffn("a", ys_a, ybs_a)
        ys_b, ybs_b = epilogue_pre("b", pvs_b)
        epilogue_ffn("b", ys_b, ybs_b)

    stack.close()


_CACHE = {}


def _build():
    if "nc" in _CACHE:
        return _CACHE["nc"]
    nc = bacc.Bacc("TRN2", target_bir_lowering=False, debug=False, num_devices=R)
    ins = {}
    for nm, shape in (("faT", [D, NS]), ("fbT", [D, NS])):
        ins[nm] = nc.dram_tensor(nm, shape, F32, kind="ExternalInput")
    for p in ("a", "b"):
        for w in WNAMES:
            ins[f"{p}_{w}"] = nc.dram_tensor(f"{p}_{w}", [D, D], F32, kind="ExternalInput")
        ins[f"{p}_ffn"] = nc.dram_tensor(f"{p}_ffn", [D, D], BF16, kind="ExternalInput")
        for b in BNAMES + ("bk", "bq"):
            ins[f"{p}_{b}"] = nc.dram_tensor(f"{p}_{b}", [P, KT], F32, kind="ExternalInput")
        ins[f"{p}_rb3bc"] = nc.dram_tensor(f"{p}_rb3bc", [P, D], F32, kind="ExternalInput")
        ins[f"{p}_bvbc"] = nc.dram_tensor(f"{p}_bvbc", [P, D], F32, kind="ExternalInput")
        for g in ("lng", "lnb"):
            ins[f"{p}_{g}"] = nc.dram_tensor(f"{p}_{g}", [P, D], F32, kind="ExternalInput")
    out_ext = nc.dram_tensor("out", [2, NS, D], F32, kind="ExternalOutput")

    with tile.TileContext(nc) as tc:
        _body(tc, {k: v.ap() for k, v in ins.items()}, out_ext.ap())
    nc.compile()
    _CACHE["nc"] = nc
    return nc


def _r32r(x):
    xi = np.ascontiguousarray(x, dtype=np.float32).view(np.uint32)
    xi = (xi + np.uint32(1 << 12)) & np.uint32(0xFFFFE000)
    return xi.view(np.float32)


_WMAP = {"pw1": "pW1", "rw1": "rW1", "rw2": "rW2", "rw3": "rW3",
         "wq": "Wq", "wk": "Wk", "wv": "Wv"}
_BMAP = {"pb1": "pb1", "pb2": "pb2", "rb1": "rb1", "rb2": "rb2"}


def _prep_in_maps(inputs):
    shared = {}
    for p, pre in (("A", "a"), ("B", "b")):
        # fold the second projection layer into the Q/K/V weights (host-side):
        # K = feat@Wk.T = h@(Wk@pW2).T + (Wk@pb2)
        pW2 = np.asarray(inputs[f"{p}_pW2"], dtype=np.float64)
        pb2 = np.asarray(inputs[f"{p}_pb2"], dtype=np.float64)
        inputs = dict(inputs)
        for dn, rn in (("wk", "Wk"), ("wq", "Wq"), ("wv", "Wv"), ("rw1", "rW1")):
            Wx = np.asarray(inputs[f"{p}_{rn}"], dtype=np.float64)
            inputs[f"{p}_{rn}_folded"] = (Wx @ pW2).astype(np.float32)
            inputs[f"{p}_{rn}_bias"] = (Wx @ pb2).astype(np.float32)
        # rw1's folded bias merges with its own bias
        inputs[f"{p}_rb1"] = (np.asarray(inputs[f"{p}_rW1_bias"], dtype=np.float64)
                              + np.asarray(inputs[f"{p}_rb1"], dtype=np.float64)).astype(np.float32)
        for dn, rn in _WMAP.items():
            src = f"{p}_{rn}_folded" if dn in ("wk", "wq", "wv", "rw1") else f"{p}_{rn}"
            shared[f"{pre}_{dn}"] = _r32r(np.ascontiguousarray(np.asarray(inputs[src]).T.astype(np.float32)))
        shared[f"{pre}_bk"] = np.ascontiguousarray(
            np.asarray(inputs[f"{p}_Wk_bias"]).reshape(KT, P).T)
        shared[f"{pre}_bq"] = np.ascontiguousarray(
            np.asarray(inputs[f"{p}_Wq_bias"]).reshape(KT, P).T)
        shared[f"{pre}_bvbc"] = np.ascontiguousarray(
            np.broadcast_to(np.asarray(inputs[f"{p}_Wv_bias"]), (P, D)).astype(np.float32))
        shared[f"{pre}_ffn"] = np.ascontiguousarray(
            np.asarray(inputs[f"{p}_ffnW"]).T.astype(np.float32)).astype(ml_dtypes.bfloat16)
        for dn, rn in _BMAP.items():
            shared[f"{pre}_{dn}"] = np.ascontiguousarray(
                np.asarray(inputs[f"{p}_{rn}"]).astype(np.float32).reshape(KT, P).T)
        shared[f"{pre}_rb3bc"] = np.ascontiguousarray(
            np.broadcast_to(np.asarray(inputs[f"{p}_rb3"]).astype(np.float32), (P, D)))
        shared[f"{pre}_lng"] = np.ascontiguousarray(
            np.broadcast_to(np.asarray(inputs[f"{p}_ln_g"]).astype(np.float32), (P, D)))
        shared[f"{pre}_lnb"] = np.ascontiguousarray(
            np.broadcast_to(np.asarray(inputs[f"{p}_ln_b"]).astype(np.float32), (P, D)))
    fa = np.asarray(inputs["feat_a"]).astype(np.float32)
    fb = np.asarray(inputs["feat_b"]).astype(np.float32)
    in_maps = []
    for r in range(R):
        m = dict(shared)
        m["faT"] = _r32r(np.ascontiguousarray(fa[r * NS:(r + 1) * NS].T))
        m["fbT"] = _r32r(np.ascontiguousarray(fb[r * NS:(r + 1) * NS].T))
        in_maps.append(m)
    return in_maps


def kernel(**inputs) -> np.ndarray:
    nc = _build()
    in_maps = _prep_in_maps(inputs)
    res = bass_utils.run_bass_kernel_spmd(nc, in_maps, core_ids=list(range(R)))
    outs = res.results
    A = np.concatenate([outs[r]["out"][0] for r in range(R)], axis=0)
    B = np.concatenate([outs[r]["out"][1] for r in range(R)], axis=0)
    return np.stack([A, B], axis=0)


# revision 54
# speedup vs baseline: 1.2387x; 1.0223x over previous
"""Trainium2 Bass kernel for the dual-branch cross-attention block (N=4096, D=512).

Sharding: rows of Q / output across 8 cores (512 rows each). K and V shards are
computed locally and all-gathered; the column-sum renorm uses a tiny AllReduce.

v2 layout/schedule:
- linears in f32r; V / P-transpose storage / FFN path in bf16 (rel_err ~5e-3).
- V and the residual-MLP output are produced directly in natural (row-major)
  layout by using the activation tiles as the matmul stationary operand.
- Row softmax scale (1/rowsum) is applied to the exp'd scores on the scalar
  engine before the P transposes; column sums then come from cheap vector
  reduces over the transposed tiles (no tensor-engine colsum matmuls).
- Branches are interleaved (S_a, S_b, PV_a+epilogue_a, PV_b+epilogue_b) so both
  column-sum AllReduces hide under tensor work; V gathers are bf16.
"""
import numpy as np

try:
    import concourse.bass as bass  # noqa: F401
except ImportError:
    import sys
    sys.path.insert(0, "/opt/trn_rl_repo")
import concourse.bass as bass
import concourse.tile as tile
from concourse import bacc, mybir
from concourse import bass_utils
from concourse.masks import make_identity
import ml_dtypes

F32 = mybir.dt.float32
F32R = mybir.dt.float32r
BF16 = mybir.dt.bfloat16
AF = mybir.ActivationFunctionType
ALU = mybir.AluOpType
AX = mybir.AxisListType

N, D, R, NS, P = 4096, 512, 8, 512, 128
KT = D // P   # 4 d-tiles
IT = NS // P  # 4 i-tiles per core
NT = N // P   # 32 n-tiles global
RG = [list(range(R))]
EPS_LN, EPS_ATTN = 1e-6, 1e-9

WNAMES = ("pw1", "rw1", "rw2", "rw3", "wq", "wk", "wv")
BNAMES = ("pb1", "pb2", "rb1", "rb2")


def _body(tc, ins, out_ext):
    nc = tc.nc

    import contextlib
    stack = contextlib.ExitStack()

    # ---- whole-kernel pools ----
    const = stack.enter_context(tc.tile_pool(name="const", bufs=1))
    dram = stack.enter_context(tc.tile_pool(name="dram", bufs=1, space="DRAM"))

    ident = const.tile([P, P], F32, tag="ident", bufs=1)
    make_identity(nc, ident[:])
    identr = const.tile([P, P], F32R, tag="identr", bufs=1)
    nc.vector.tensor_copy(identr[:], ident[:])
    identb = const.tile([P, P], BF16, tag="identb", bufs=1)
    nc.vector.tensor_copy(identb[:], ident[:])
    onesb = const.tile([P, 1], BF16, tag="onesb", bufs=1)
    nc.vector.memset(onesb[:], 1.0)
    epsln = const.tile([P, 1], F32, tag="epsln", bufs=1)
    nc.vector.memset(epsln[:], EPS_LN)

    # LN params (pre-replicated on host to [128, 512]); DMAs deferred past the
    # startup-critical x0/weight loads (lnp only needed in the epilogue).
    lnp = {}
    for p in ("a", "b"):
        for g in ("lng", "lnb"):
            lnp[p, g] = const.tile([P, D], F32, tag="lnp", bufs=4, name=f"lnp_{p}_{g}")
    # feature-major biases [128, 4]; rb3 broadcast [128, 512]
    bias = {}
    for p in ("a", "b"):
        for b in BNAMES + ("bk", "bq"):
            t = const.tile([P, KT], F32, tag="bias", bufs=12, name=f"bias_{p}_{b}")
            # scalar DMA queue: keeps 12 tiny trigger slots off the sync queue
            # head, ahead of the startup-critical x0/weight stream
            nc.scalar.dma_start(t[:], ins[f"{p}_{b}"][:, :])
            bias[p, b] = t
        bias[p, "rb3bc"] = const.tile([P, D], F32, tag="b3bc", bufs=4, name=f"b3bc_{p}")
        bias[p, "bvbc"] = const.tile([P, D], F32, tag="b3bc", bufs=4, name=f"bvbc_{p}")

    # long-lived activations
    qT = {}
    res_nat = {}
    pts = {}
    for p in ("a", "b"):
        qT[p] = const.tile([P, KT, NS], BF16, tag="qT", bufs=2, name=f"qT_{p}")
        res_nat[p] = const.tile([P, IT, D], F32, tag="resnat", bufs=2, name=f"resnat_{p}")
        pts[p] = const.tile([P, NT, NS], BF16, tag="pts", bufs=2, name=f"pts_{p}")

    # DRAM bounce buffers for collectives
    kag_in = {p: dram.tile([D, NS], BF16, tag=f"kag_in_{p}", name=f"kag_in_{p}") for p in ("a", "b")}
    kag_out = {p: dram.tile([R, D, NS], BF16, tag=f"kag_out_{p}", name=f"kag_out_{p}", addr_space="Shared") for p in ("a", "b")}
    vag_in = {p: dram.tile([NS, D], BF16, tag=f"vag_in_{p}", name=f"vag_in_{p}") for p in ("a", "b")}
    vag_out = {p: dram.tile([R, NS, D], BF16, tag=f"vag_out_{p}", name=f"vag_out_{p}", addr_space="Shared") for p in ("a", "b")}
    cs_in = {p: dram.tile([1, N], F32, tag=f"cs_in_{p}", name=f"cs_in_{p}") for p in ("a", "b")}
    cs_out = {p: dram.tile([1, N], F32, tag=f"cs_out_{p}", name=f"cs_out_{p}", addr_space="Shared") for p in ("a", "b")}

    # ================= stage 1+2: projections, K/V (+gathers), Q, residual =================
    with tc.tile_pool(name="lin", bufs=1) as lin, \
         tc.tile_pool(name="ps12", bufs=1, space="PSUM") as ps12:

        def load_w(wname):
            # per-k-chunk DMAs: matmuls can start on the first 256KB instead of
            # waiting for the full 1MB, and the stream interleaves more finely
            # with collective traffic
            w = lin.tile([P, KT, D], F32R, tag="w", bufs=3, name=f"w_{wname}")
            src = ins[wname].rearrange("(k p) o -> p k o", p=P).bitcast(F32R)
            for k in range(KT):
                nc.sync.dma_start(w[:, k, :], src[:, k, :])
            return w

        def linT(x, wname, bias_t=None, func=AF.Copy, out_dtype=F32R, tag="act", bufs=4, name=None, out=None):
            """Feature-major linear: y^T = func(W @ x^T + b); x, y: [128, KT, NS]."""
            w = load_w(wname)
            y = out if out is not None else lin.tile([P, KT, NS], out_dtype, tag=tag, bufs=bufs, name=name or f"y_{wname}")
            for j in range(KT):
                pm = ps12.tile([P, NS], F32, tag="lin", bufs=4, name="pm_lin")
                for k in range(KT):
                    nc.tensor.matmul(pm[:], w[:, k, j * P:(j + 1) * P], x[:, k, :],
                                     start=(k == 0), stop=(k == KT - 1))
                bias_ap = bias_t[:, j:j + 1] if bias_t is not None else 0.0
                nc.scalar.activation(y[:, j, :], pm[:], AF.Identity if func == AF.Copy and bias_t is not None else func,
                                     bias=bias_ap)
            return y

        def linN(x, wname, out_nat, bias_bcast=None):
            """Row-major linear: out_nat[:, it, :] = x^T_it @ W^T (+b); x feature-major f32r,
            out_nat [128, IT, D] (dtype from tile)."""
            w = load_w(wname)
            for it in range(IT):
                pm = ps12.tile([P, D], F32, tag="nat", bufs=2, name="pm_nat")
                for k in range(KT):
                    nc.tensor.matmul(pm[:], x[:, k, it * P:(it + 1) * P], w[:, k, :],
                                     start=(k == 0), stop=(k == KT - 1))
                if bias_bcast is not None:
                    nc.vector.tensor_add(pm[:], pm[:], bias_bcast[:])
                nc.scalar.activation(out_nat[:, it, :], pm[:], AF.Copy)

        # K/V/Q weights are pre-folded through pW2 on the host, so they consume
        # the first-layer activations h directly — the K gathers launch one
        # matmul-layer earlier. Branch b first so the K_b gather leads.
        hact = {}
        for p in ("b", "a"):
            x0 = lin.tile([P, KT, NS], F32R, tag="fin", bufs=2, name=f"fin_{p}")
            x0src = ins["faT" if p == "a" else "fbT"].rearrange("(k p) i -> p k i", p=P).bitcast(F32R)
            for k in range(KT):
                nc.sync.dma_start(x0[:, k, :], x0src[:, k, :])
            hact[p] = linT(x0, f"{p}_pw1", bias[p, "pb1"], AF.Relu, tag="h", bufs=3, name=f"h_{p}")
            kT = linT(hact[p], f"{p}_wk", bias[p, "bk"], AF.Identity, out_dtype=BF16, tag="kv", bufs=2, name=f"kT_{p}")
            for k in range(KT):
                nc.sync.dma_start(kag_in[p][k * P:(k + 1) * P, :], kT[:, k, :])
            nc.gpsimd.collective_compute("AllGather", ALU.bypass, replica_groups=RG,
                                         ins=[kag_in[p].opt()], outs=[kag_out[p].opt()])

        # deferred constants via the scalar (Activation) DMA queue, off the
        # startup-critical path but BEFORE their first consumers (bvbc feeds
        # the V projection below)
        for p in ("a", "b"):
            nc.scalar.dma_start(bias[p, "bvbc"][:], ins[f"{p}_bvbc"][:, :])
            nc.scalar.dma_start(bias[p, "rb3bc"][:], ins[f"{p}_rb3bc"][:, :])
            for g in ("lng", "lnb"):
                nc.scalar.dma_start(lnp[p, g][:], ins[f"{p}_{g}"][:, :])

        # V in natural layout (bf16), gathered in need-order (V_a for PV_a first)
        for p in ("a", "b"):
            v_nat = lin.tile([P, IT, D], BF16, tag="vnat", bufs=2, name=f"vnat_{p}")
            linN(hact[p], f"{p}_wv", v_nat, bias_bcast=bias[p, "bvbc"])
            for it in range(IT):
                nc.sync.dma_start(vag_in[p][it * P:(it + 1) * P, :], v_nat[:, it, :])
            nc.gpsimd.collective_compute("AllGather", ALU.bypass, replica_groups=RG,
                                         ins=[vag_in[p].opt()], outs=[vag_out[p].opt()])

        # local compute that overlaps the gathers; rw1 is also pre-folded
        # through pW2 on the host, so feat is never materialized
        for p in ("a", "b"):
            linT(hact[p], f"{p}_wq", bias[p, "bq"], AF.Identity, out=qT[p])
        for p in ("a", "b"):
            r1 = linT(hact[p], f"{p}_rw1", bias[p, "rb1"], AF.Relu, tag="h", bufs=3)
            r2 = linT(r1, f"{p}_rw2", bias[p, "rb2"], AF.Relu, tag="h", bufs=3)
            linN(r2, f"{p}_rw3", res_nat[p], bias_bcast=bias[p, "rb3bc"])

    # ================= attention scores + online softmax + P^T, per branch =================
    rcs = {}
    for p in ("a", "b"):
        rcs[p] = const.tile([P, NT], F32, tag="rcs", bufs=2, name=f"rcs_{p}")

    def attn_scores(p, o, mid_hook=None):
        with tc.tile_pool(name=f"s_{p}", bufs=1) as sp, \
             tc.tile_pool(name=f"psS_{p}", bufs=1, space="PSUM") as ps:
            # bf16 storage of exp'd scores is safe: values <= 1 after the
            # running-max subtraction; logits are consumed straight from PSUM f32.
            sgb = [sp.tile([P, N], BF16, tag="sg", bufs=IT, name=f"sg_{p}_{it}") for it in range(IT)]
            m_hist = [sp.tile([P, R], F32, tag="mh", bufs=IT, name=f"mh_{p}_{it}") for it in range(IT)]
            negm_h = [sp.tile([P, R], F32, tag="nh", bufs=IT, name=f"nh_{p}_{it}") for it in range(IT)]
            s_hist = [sp.tile([P, R], F32, tag="sh", bufs=IT, name=f"sh_{p}_{it}") for it in range(IT)]
            csf = sp.tile([1, N], F32, tag="csf", bufs=1, name=f"csf_{p}")
            for r in range(R):
                kch = sp.tile([P, KT, NS], BF16, tag="kch", bufs=R, name=f"kch_{p}")
                ksrc = kag_out[o][r, :, :].rearrange("(k p) i -> p k i", p=P)
                if r == 0:
                    # per-k split so the first S matmul starts on 128KB
                    for k in range(KT):
                        nc.sync.dma_start(kch[:, k, :], ksrc[:, k, :])
                else:
                    nc.sync.dma_start(kch[:], ksrc)
                for it in range(IT):
                    pm = ps.tile([P, NS], F32, tag="s", bufs=4, name="pm_s")
                    for k in range(KT):
                        nc.tensor.matmul(pm[:], qT[p][:, k, it * P:(it + 1) * P], kch[:, k, :],
                                         start=(k == 0), stop=(k == KT - 1))
                    if r == 0:
                        nc.vector.tensor_reduce(m_hist[it][:, 0:1], pm[:], AX.X, ALU.max)
                    else:
                        mxc = sp.tile([P, 1], F32, tag="mxc", bufs=4, name="mxc")
                        nc.vector.tensor_reduce(mxc[:], pm[:], AX.X, ALU.max)
                        nc.vector.tensor_tensor(m_hist[it][:, r:r + 1], m_hist[it][:, r - 1:r], mxc[:], ALU.max)
                    nc.vector.tensor_scalar_mul(negm_h[it][:, r:r + 1], m_hist[it][:, r:r + 1], -1.0)
                    # exp straight out of PSUM (f32 logits), bf16 store, chunk sum accumulated
                    nc.scalar.activation(sgb[it][:, r * NS:(r + 1) * NS], pm[:], AF.Exp,
                                         bias=negm_h[it][:, r:r + 1], accum_out=s_hist[it][:, r:r + 1])
            # correction factors exp(m_r - m_final); rowsum = sum_r s_r * fac_r.
            # Batched per engine to minimize cross-engine round trips.
            fac = [sp.tile([P, R], F32, tag="fac", bufs=IT, name=f"fac_{it}") for it in range(IT)]
            sf = [sp.tile([P, R], F32, tag="sf", bufs=IT, name=f"sf_{it}") for it in range(IT)]
            rlf = [sp.tile([P, 1], F32, tag="rlf", bufs=IT, name=f"rlf_{it}") for it in range(IT)]
            for it in range(IT):
                nc.scalar.activation(fac[it][:], m_hist[it][:], AF.Exp, bias=negm_h[it][:, R - 1:R])
            for it in range(IT):
                nc.vector.tensor_tensor(sf[it][:], s_hist[it][:], fac[it][:], ALU.mult)
                nc.vector.tensor_reduce(rlf[it][:], sf[it][:], AX.X, ALU.add)
                nc.vector.reciprocal(rlf[it][:], rlf[it][:])
            for it in range(IT):
                nc.scalar.activation(fac[it][:], fac[it][:], AF.Copy, scale=rlf[it][:])
            if mid_hook is not None:
                mid_hook()
            # normalization scales r-major (split scalar/vector), each chunk's
            # column-sum matmuls issued right behind its scales so the AllReduce
            # input is complete ~when the scale pass ends
            for r in range(R):
                for it in range(IT):
                    sl = sgb[it][:, r * NS:(r + 1) * NS]
                    if it % 2 == 0:
                        nc.scalar.activation(sl, sl, AF.Copy, scale=fac[it][:, r:r + 1])
                    else:
                        nc.vector.tensor_scalar_mul(sl, sl, fac[it][:, r:r + 1])
                pc = ps.tile([1, NS], F32, tag="col", bufs=2, name="pm_col")
                for it in range(IT):
                    nc.tensor.matmul(pc[:], onesb[:], sgb[it][:, r * NS:(r + 1) * NS],
                                     start=(it == 0), stop=(it == IT - 1))
                nc.scalar.activation(csf[:, r * NS:(r + 1) * NS], pc[:], AF.Copy)
            # P^T tiles: 4 transposes share one PSUM bank -> one strided copy
            for it in range(IT):
                for tq in range(NT // 4):
                    pm = ps.tile([P, 4, P], BF16, tag="tp", bufs=2, name="pm_tp")
                    for tt in range(4):
                        nc.tensor.transpose(pm[:, tt, :], sgb[it][:, (4 * tq + tt) * P:(4 * tq + tt + 1) * P],
                                            identb[:])
                    if tq % 2 == 0:
                        nc.vector.tensor_copy(pts[p][:, 4 * tq:4 * tq + 4, it * P:(it + 1) * P], pm[:])
                    else:
                        nc.scalar.activation(pts[p][:, 4 * tq:4 * tq + 4, it * P:(it + 1) * P], pm[:], AF.Copy)
            # contiguous 16KB column-sum payload on the (idle) gpsimd queue
            nc.gpsimd.dma_start(cs_in[p][:, :], csf[:, :])
            nc.gpsimd.collective_compute("AllReduce", ALU.add, replica_groups=RG,
                                         ins=[cs_in[p].opt()], outs=[cs_out[p].opt()])
            nc.gpsimd.dma_start(rcs[p][:], cs_out[p][:, :].rearrange("o (t p) -> p (o t)", p=P))

    def _recip_hook(p):
        # 1/(eps + colsum): tiny vector ops slotted early in the other branch's
        # vector stream, right after that branch's fac math
        def h():
            nc.vector.tensor_scalar_add(rcs[p][:], rcs[p][:], EPS_ATTN)
            nc.vector.reciprocal(rcs[p][:], rcs[p][:])
        return h

    attn_scores("a", "b")
    attn_scores("b", "a", mid_hook=_recip_hook("a"))

    # ================= PV + epilogue, branches interleaved =================
    with tc.tile_pool(name="tail", bufs=1) as tail, \
         tc.tile_pool(name="psB", bufs=1, space="PSUM") as psB:

        ffnw = {}
        vchs = {}
        for p in ("a", "b"):
            ffnw[p] = tail.tile([P, KT, D], BF16, tag="ffnw", bufs=2, name=f"ffnw_{p}")
            nc.sync.dma_start(ffnw[p][:], ins[f"{p}_ffn"].rearrange("(k p) o -> p k o", p=P))
        # prefetch ALL V chunks for both branches (deps: the V all-gathers only)
        for p in ("a", "b"):
            vchs[p] = [tail.tile([P, IT, D], BF16, tag="vch", bufs=11, name=f"vch_{p}_{r}")
                       for r in range(R)]
            for r in range(R):
                nc.sync.dma_start(vchs[p][r][:], vag_out[p][r, :, :].rearrange("(j p) d -> p j d", p=P))

        def pv_prep(p):
            """V-chunk scaling by rcs, split across scalar and vector queues
            (reciprocal for branch a was already slotted into branch b's
            attention; do branch b's here)."""
            if p == "b":
                nc.vector.tensor_scalar_add(rcs[p][:], rcs[p][:], EPS_ATTN)
                nc.vector.reciprocal(rcs[p][:], rcs[p][:])
            for r in range(R):
                for jj in range(IT):
                    t = IT * r + jj
                    if jj % 2 == 0:
                        nc.scalar.activation(vchs[p][r][:, jj, :], vchs[p][r][:, jj, :],
                                             AF.Copy, scale=rcs[p][:, t:t + 1])
                    else:
                        nc.vector.tensor_scalar_mul(vchs[p][r][:, jj, :], vchs[p][r][:, jj, :],
                                                    rcs[p][:, t:t + 1])

        def pv_mm(p):
            # it-major: each output tile's accumulation completes early, so the
            # LN -> transpose -> FFN chain pipelines behind the remaining matmuls
            pvs = [psB.tile([P, D], F32, tag="pv", bufs=IT, name=f"pm_pv_{p}_{it}") for it in range(IT)]
            for it in range(IT):
                for r in range(R):
                    vch = vchs[p][r]
                    for jj in range(IT):
                        t = IT * r + jj
                        nc.tensor.matmul(pvs[it][:], pts[p][:, t, it * P:(it + 1) * P], vch[:, jj, :],
                                         start=(r == 0 and jj == 0), stop=(r == R - 1 and jj == IT - 1))
            return pvs

        def epilogue_pre(p, pvs):
            """PSUM drain + add residual + LN -> y (f32) and yb (bf16)."""
            ys, ybs = [], []
            for it in range(IT):
                x = tail.tile([P, D], F32, tag="x", bufs=4, name=f"x_{p}_{it}")
                nc.vector.tensor_add(x[:], pvs[it][:], res_nat[p][:, it, :])
                negmu = tail.tile([P, 1], F32, tag="negmu", bufs=4, name=f"negmu_{it}")
                nc.vector.tensor_reduce(negmu[:], x[:], AX.X, ALU.add, negate=True)
                nc.scalar.mul(negmu[:], negmu[:], 1.0 / D)
                xc = tail.tile([P, D], F32, tag="xc", bufs=2, name=f"xc_{it}")
                nc.scalar.add(xc[:], x[:], negmu[:])
                scr = tail.tile([P, D], F32, tag="x", bufs=4, name=f"scr_{it}")
                ssq = tail.tile([P, 1], F32, tag="ssq", bufs=4, name=f"ssq_{it}")
                nc.scalar.activation(scr[:], xc[:], AF.Square, accum_out=ssq[:])
                std = tail.tile([P, 1], F32, tag="std", bufs=4, name=f"std_{it}")
                nc.vector.tensor_scalar(std[:], ssq[:], 1.0 / D, EPS_LN, ALU.mult, ALU.add)
                nc.scalar.sqrt(std[:], std[:])
                rstd = tail.tile([P, 1], F32, tag="rstd", bufs=4, name=f"rstd_{it}")
                nc.vector.reciprocal(rstd[:], std[:])
                y = tail.tile([P, D], F32, tag="y", bufs=2 * IT, name=f"y_{p}_{it}")
                nc.vector.scalar_tensor_tensor(y[:], xc[:], rstd[:], lnp[p, "lng"][:],
                                               op0=ALU.mult, op1=ALU.mult)
                nc.vector.tensor_add(y[:], y[:], lnp[p, "lnb"][:])
                yb = tail.tile([P, D], BF16, tag="yb", bufs=2 * IT, name=f"yb_{p}_{it}")
                nc.vector.tensor_copy(yb[:], y[:])
                ys.append(y)
                ybs.append(yb)
            return ys, ybs

        def epilogue_ffn(p, ys, ybs):
            bi = 0 if p == "a" else 1
            asT = tail.tile([P, KT, NS], BF16, tag="asT", bufs=2, name=f"asT_{p}")
            for it in range(IT):
                pm = psB.tile([P, KT, P], BF16, tag="tp", bufs=2, name="pm_tpb")
                for k in range(KT):
                    nc.tensor.transpose(pm[:, k, :], ybs[it][:, k * P:(k + 1) * P], identb[:])
                nc.vector.tensor_copy(asT[:, 0:KT, it * P:(it + 1) * P], pm[:])
            for it in range(IT):
                pf = psB.tile([P, D], F32, tag="ffn", bufs=2, name="pm_ffn")
                for k in range(KT):
                    nc.tensor.matmul(pf[:], asT[:, k, it * P:(it + 1) * P], ffnw[p][:, k, :],
                                     start=(k == 0), stop=(k == KT - 1))
                outt = tail.tile([P, D], F32, tag="outt", bufs=2, name=f"outt_{it}")
                nc.vector.tensor_add(outt[:], ys[it][:], pf[:])
                nc.sync.dma_start(out_ext[bi, it * P:(it + 1) * P, :], outt[:])

        pv_prep("a")
        pvs_a = pv_mm("a")
        ys_a, ybs_a = epilogue_pre("a", pvs_a)   # frees pvs_a banks for pv_b
        pv_prep("b")
        pvs_b = pv_mm("b")                        # tensor works here while LN_a runs
        epilogue_ffn("a", ys_a, ybs_a)
        ys_b, ybs_b = epilogue_pre("b", pvs_b)
        epilogue_ffn("b", ys_b, ybs_b)

    stack.close()


_CACHE = {}


def _build():
    if "nc" in _CACHE:
        return _CACHE["nc"]
    nc = bacc.Bacc("TRN2", target_bir_lowering=False, debug=False, num_devices=R)
    ins = {}
    for nm, shape in (("faT", [D, NS]), ("fbT", [D, NS])):
        ins[nm] = nc.dram_tensor(nm, shape, F32, kind="ExternalInput")
    for p in ("a", "b"):
        for w in WNAMES:
            ins[f"{p}_{w}"] = nc.dram_tensor(f"{p}_{w}", [D, D], F32, kind="ExternalInput")
        ins[f"{p}_ffn"] = nc.dram_tensor(f"{p}_ffn", [D, D], BF16, kind="ExternalInput")
        for b in BNAMES + ("bk", "bq"):
            ins[f"{p}_{b}"] = nc.dram_tensor(f"{p}_{b}", [P, KT], F32, kind="ExternalInput")
        ins[f"{p}_rb3bc"] = nc.dram_tensor(f"{p}_rb3bc", [P, D], F32, kind="ExternalInput")
        ins[f"{p}_bvbc"] = nc.dram_tensor(f"{p}_bvbc", [P, D], F32, kind="ExternalInput")
        for g in ("lng", "lnb"):
            ins[f"{p}_{g}"] = nc.dram_tensor(f"{p}_{g}", [P, D], F32, kind="ExternalInput")
    out_ext = nc.dram_tensor("out", [2, NS, D], F32, kind="ExternalOutput")

    with tile.TileContext(nc) as tc:
        _body(tc, {k: v.ap() for k, v in ins.items()}, out_ext.ap())
    nc.compile()
    _CACHE["nc"] = nc
    return nc


def _r32r(x):
    xi = np.ascontiguousarray(x, dtype=np.float32).view(np.uint32)
    xi = (xi + np.uint32(1 << 12)) & np.uint32(0xFFFFE000)
    return xi.view(np.float32)


_WMAP = {"pw1": "pW1", "rw1": "rW1", "rw2": "rW2", "rw3": "rW3",
         "wq": "Wq", "wk": "Wk", "wv": "Wv"}
_BMAP = {"pb1": "pb1", "pb2": "pb2", "rb1": "rb1", "rb2": "rb2"}


def _prep_in_maps(inputs):
    shared = {}
    for p, pre in (("A", "a"), ("B", "b")):
        # fold the second projection layer into the Q/K/V weights (host-side):
        # K = feat@Wk.T = h@(Wk@pW2).T + (Wk@pb2)
        pW2 = np.asarray(inputs[f"{p}_pW2"], dtype=np.float64)
        pb2 = np.asarray(inputs[f"{p}_pb2"], dtype=np.float64)
        inputs = dict(inputs)
        for dn, rn in (("wk", "Wk"), ("wq", "Wq"), ("wv", "Wv"), ("rw1", "rW1")):
            Wx = np.asarray(inputs[f"{p}_{rn}"], dtype=np.float64)
            inputs[f"{p}_{rn}_folded"] = (Wx @ pW2).astype(np.float32)
            inputs[f"{p}_{rn}_bias"] = (Wx @ pb2).astype(np.float32)
        # rw1's folded bias merges with its own bias
        inputs[f"{p}_rb1"] = (np.asarray(inputs[f"{p}_rW1_bias"], dtype=np.float64)
                              + np.asarray(inputs[f"{p}_rb1"], dtype=np.float64)).astype(np.float32)
        for dn, rn in _WMAP.items():
            src = f"{p}_{rn}_folded" if dn in ("wk", "wq", "wv", "rw1") else f"{p}_{rn}"
            shared[f"{pre}_{dn}"] = _r32r(np.ascontiguousarray(np.asarray(inputs[src]).T.astype(np.float32)))
        shared[f"{pre}_bk"] = np.ascontiguousarray(
            np.asarray(inputs[f"{p}_Wk_bias"]).reshape(KT, P).T)
        shared[f"{pre}_bq"] = np.ascontiguousarray(
            np.asarray(inputs[f"{p}_Wq_bias"]).reshape(KT, P).T)
        shared[f"{pre}_bvbc"] = np.ascontiguousarray(
            np.broadcast_to(np.asarray(inputs[f"{p}_Wv_bias"]), (P, D)).astype(np.float32))
        shared[f"{pre}_ffn"] = np.ascontiguousarray(
            np.asarray(inputs[f"{p}_ffnW"]).T.astype(np.float32)).astype(ml_dtypes.bfloat16)
        for dn, rn in _BMAP.items():
            shared[f"{pre}_{dn}"] = np.ascontiguousarray(
                np.asarray(inputs[f"{p}_{rn}"]).astype(np.float32).reshape(KT, P).T)
        shared[f"{pre}_rb3bc"] = np.ascontiguousarray(
            np.broadcast_to(np.asarray(inputs[f"{p}_rb3"]).astype(np.float32), (P, D)))
        shared[f"{pre}_lng"] = np.ascontiguousarray(
            np.broadcast_to(np.asarray(inputs[f"{p}_ln_g"]).astype(np.float32), (P, D)))
        shared[f"{pre}_lnb"] = np.ascontiguousarray(
            np.broadcast_to(np.asarray(inputs[f"{p}_ln_b"]).astype(np.float32), (P, D)))
    fa = np.asarray(inputs["feat_a"]).astype(np.float32)
    fb = np.asarray(inputs["feat_b"]).astype(np.float32)
    in_maps = []
    for r in range(R):
        m = dict(shared)
        m["faT"] = _r32r(np.ascontiguousarray(fa[r * NS:(r + 1) * NS].T))
        m["fbT"] = _r32r(np.ascontiguousarray(fb[r * NS:(r + 1) * NS].T))
        in_maps.append(m)
    return in_maps


def kernel(**inputs) -> np.ndarray:
    nc = _build()
    in_maps = _prep_in_maps(inputs)
    res = bass_utils.run_bass_kernel_spmd(nc, in_maps, core_ids=list(range(R)))
    outs = res.results
    A = np.concatenate([outs[r]["out"][0] for r in range(R)], axis=0)
    B = np.concatenate([outs[r]["out"][1] for r in range(R)], axis=0)
    return np.stack([A, B], axis=0)
